# revision 8
# baseline (speedup 1.0000x reference)
"""Trainium2 Bass kernel for the MulT-style cross-modal CNN/transformer.

Strategy (SPMD over 8 NeuronCores):
  - The model contains 4 independent 5-layer cross-modal encoders followed by
    4 independent conv branches and a tiny shared head. Core c (c in 0..3)
    runs encoder c and conv branch c on its own encoder output; cores 4..7
    mirror cores 0..3 (same program, same data -> harmless redundancy).
  - Branch features (1,10) are AllGather'd within {0,1,2,3} (and {4,5,6,7}),
    and the 2-layer sigmoid head is computed redundantly on every core.
  - Host-side prep folds all LayerNorm gains/biases into the adjacent weight
    matrices, folds all biases into ones-row augmented matmuls, pre-applies
    the attention output projection to V (vo = v @ Wo + bo), and packs every
    constant this core needs into ONE [128, NCOL] f32 tensor -> one DMA.
  - On-device transcendentals use only the `natural_log_exp_and_others`
    activation table (copy/relu/ln/exp): rsqrt(v) = exp(-0.5*ln(v+eps)),
    sigmoid(x) = 1/(1+exp(-x)) -> exactly one ACT table load per run.
  - All SBUF/PSUM access patterns start at partition 0 (HW requires
    quadrant-aligned partition offsets). Ones-augmentation rows are produced
    by transposing a ones-column or by an extra unit column in the
    stationary weights; small-vector bias adds use extra accumulating
    matmuls against a [1,1] ones tile.
"""

import numpy as np
from contextlib import ExitStack

import concourse.bass as bass
import concourse.tile as tile
from concourse import bacc, mybir
from concourse.bass_utils import run_bass_kernel_spmd

F32 = mybir.dt.float32
AF = mybir.ActivationFunctionType
ALU = mybir.AluOpType
AX = mybir.AxisListType

SEQ, D, L, FF, OC, NB = 123, 16, 5, 64, 10, 4
WIN, TD = 140, 16
KS = (8, 9, 10)            # conv kernel sizes
NTAP = max(KS)             # 10 accumulation taps
EPS = 1e-5
SCL = float(D) ** 0.5      # 4.0
ISC = float(D) ** -0.5     # 0.25

# ------------------------------------------------------------------ layout --
# One [128, NCOL] f32 constants tensor per core. Column allocator:
_cols: dict[str, tuple[int, int]] = {}
_ncol = 0


def _alloc(name: str, cols: int) -> None:
    global _ncol
    _cols[name] = (_ncol, cols)
    _ncol += cols


_alloc("ident", SEQ)           # [123,123] identity (rows 0..122)
_alloc("xq", D)                # [123,16] raw query-stream input
_alloc("xk", D)                # [123,16] raw key-stream input
_alloc("pos", D)               # [123,16] positional embedding
_alloc("lnfg", D)              # [123,16] final-LN gain, broadcast
_alloc("lnfb", D)              # [123,16] final-LN bias, broadcast
for l in range(L):
    _alloc(f"wqbT{l}", D + 1)  # [16,17]  (Wq' ; bq')^T
    _alloc(f"wkb{l}", D)       # [17,16]
    _alloc(f"wvb{l}", D + 1)   # [17,17]  (Wv' ; bv') + unit col -> ones row
    _alloc(f"wob{l}", D)       # [17,16]
    _alloc(f"fc1b{l}", FF + 1)  # [17,65] (fc1';b1') + unit col -> ones row
    _alloc(f"fc2b{l}", D)      # [65,16]
for t in range(NTAP):
    _alloc(f"cw{t}", 3 * OC)   # [17,30] conv tap t, all 3 convs (+bias on t=0)
for j in range(3):
    _alloc(f"fc1w{j}", 30)     # [10,30] branch fc1 weight rows j*10..j*10+9
_alloc("fc1bb", 30)            # [1,30] branch fc1 bias
_alloc("fc2w", OC)             # [30,10]
_alloc("fc2bb", OC)            # [1,10]
_alloc("w1", 40)               # [40,40]
_alloc("w1bb", 40)             # [1,40]
_alloc("w2", 2)                # [40,2]
_alloc("w2bb", 2)              # [1,2]
NCOL = _ncol

E16 = np.zeros((17, 1))
E16[16, 0] = 1.0


def _pos_embed() -> np.ndarray:
    half = D // 2
    freqs = np.exp(np.arange(half) * (-np.log(10000.0) / (half - 1)))
    ang = np.arange(SEQ)[:, None] * freqs[None, :]
    return np.concatenate([np.sin(ang), np.cos(ang)], axis=1).astype(np.float32)


def _pack_core(xq: np.ndarray, xk: np.ndarray, enc: dict, conv: dict,
               head: dict) -> np.ndarray:
    """Build the [128, NCOL] constants tensor for one core (float64 folding)."""
    cp = np.zeros((128, NCOL), np.float64)

    def put(name: str, arr: np.ndarray) -> None:
        c0, w = _cols[name]
        assert arr.shape[1] == w, (name, arr.shape, w)
        cp[: arr.shape[0], c0:c0 + w] = arr

    put("ident", np.eye(SEQ))
    put("xq", xq.astype(np.float64))
    put("xk", xk.astype(np.float64))
    put("pos", _pos_embed().astype(np.float64))
    put("lnfg", np.broadcast_to(enc["lnf_g"], (SEQ, D)))
    put("lnfb", np.broadcast_to(enc["lnf_b"], (SEQ, D)))
    for l in range(L):
        g0, b0 = enc["ln0_g"][l], enc["ln0_b"][l]
        gk, bk_ = enc["ln0k_g"][l], enc["ln0k_b"][l]
        g1, b1 = enc["ln1_g"][l], enc["ln1_b"][l]
        wq, wk, wv, wo = enc["Wq"][l], enc["Wk"][l], enc["Wv"][l], enc["Wo"][l]
        wqb = np.vstack([g0[:, None] * wq * ISC,
                         (b0 @ wq + enc["bq"][l])[None] * ISC])       # [17,16]
        put(f"wqbT{l}", wqb.T)                                        # [16,17]
        put(f"wkb{l}", np.vstack([gk[:, None] * wk,
                                  (bk_ @ wk + enc["bk"][l])[None]]))
        wvb = np.vstack([gk[:, None] * wv, (bk_ @ wv + enc["bv"][l])[None]])
        put(f"wvb{l}", np.hstack([wvb, E16]))                         # [17,17]
        put(f"wob{l}", np.vstack([wo, enc["bo"][l][None]]))
        f1 = np.vstack([g1[:, None] * enc["fc1_w"][l],
                        (b1 @ enc["fc1_w"][l] + enc["fc1_b"][l])[None]])
        put(f"fc1b{l}", np.hstack([f1, E16]))                         # [17,65]
        put(f"fc2b{l}", np.vstack([enc["fc2_w"][l], enc["fc2_b"][l][None]]))
    # conv taps: cw_t[ic, j*10+oc] = w_j[oc, ic, t] (0 when t >= k_j);
    # bias row 16 only on tap 0 (multiplied by the enc ones-row).
    for t in range(NTAP):
        cw = np.zeros((17, 3 * OC), np.float64)
        for j, k in enumerate(KS):
            if t < k:
                cw[:D, j * OC:(j + 1) * OC] = conv[f"w{j+1}"][:, :, t].T
        if t == 0:
            cw[D, :] = np.concatenate([conv["b1"], conv["b2"], conv["b3"]])
        put(f"cw{t}", cw)
    for j in range(3):
        put(f"fc1w{j}", conv["fc1_w"][j * OC:(j + 1) * OC])
    put("fc1bb", conv["fc1_b"][None])
    put("fc2w", conv["fc2_w"])
    put("fc2bb", conv["fc2_b"][None])
    put("w1", head["w1"])
    put("w1bb", head["b1"][None])
    put("w2", head["w2"])
    put("w2bb", head["b2"][None])
    return np.ascontiguousarray(cp, np.float32)


# ------------------------------------------------------------- device build --
def _ln_block(nc, sb, st, x_ap, out_ap, tag, eps_ap):
    """LayerNorm standardize: out = (x - mean(x)) * rsqrt(var(x) + eps).
    Gains/biases are folded into downstream weights by the host."""
    negmean = st.tile([SEQ, 1], F32, tag=f"nm{tag}", name=f"nm{tag}")
    scr = sb.tile([SEQ, D], F32, tag="scr", bufs=2, name=f"scr{tag}")
    # ACT: out = -x/16 (scratch), accum = sum(-x/16) = -mean
    nc.scalar.activation(scr[:], x_ap, AF.Copy, scale=-1.0 / D,
                         accum_out=negmean[:])
    ssq = st.tile([SEQ, 1], F32, tag=f"sq{tag}", name=f"sq{tag}")
    scr2 = sb.tile([SEQ, D], F32, tag="scr", bufs=2, name=f"scr2{tag}")
    # DVE: out = (x - mean) * x, accum = sum((x-m)x) = sum((x-m)^2)
    nc.vector.scalar_tensor_tensor(scr2[:], x_ap, negmean[:], x_ap,
                                   op0=ALU.add, op1=ALU.mult,
                                   accum_out=ssq[:])
    lnv = st.tile([SEQ, 1], F32, tag=f"lv{tag}", name=f"lv{tag}")
    nc.scalar.activation(lnv[:], ssq[:], AF.Ln, scale=1.0 / D, bias=eps_ap)
    rstd = st.tile([SEQ, 1], F32, tag=f"rs{tag}", name=f"rs{tag}")
    nc.scalar.activation(rstd[:], lnv[:], AF.Exp, scale=-0.5)
    # DVE: out = (x + negmean) * rstd
    nc.vector.tensor_scalar(out_ap, x_ap, negmean[:], rstd[:],
                            op0=ALU.add, op1=ALU.mult)


def _build_nc():
    nc = bacc.Bacc("TRN2", target_bir_lowering=False, debug=False,
                   num_devices=8)
    cp_d = nc.dram_tensor("cpack", [128, NCOL], F32, kind="ExternalInput")
    attn_d = nc.dram_tensor("attn", [SEQ, SEQ], F32, kind="ExternalOutput")
    fch_d = nc.dram_tensor("fch", [FF, SEQ], F32, kind="ExternalOutput")
    fco_d = nc.dram_tensor("fco", [SEQ, D], F32, kind="ExternalOutput")
    enc_d = nc.dram_tensor("enc", [SEQ, D], F32, kind="ExternalOutput")
    head_d = nc.dram_tensor("head", [1, 2], F32, kind="ExternalOutput")

    with tile.TileContext(nc, trace_sim=False) as tc, ExitStack() as ctx:
        const = ctx.enter_context(tc.tile_pool(name="const", bufs=1))
        pers = ctx.enter_context(tc.tile_pool(name="pers", bufs=1))
        sb = ctx.enter_context(tc.tile_pool(name="sb", bufs=2))
        st = ctx.enter_context(tc.tile_pool(name="st", bufs=2))
        ps = ctx.enter_context(tc.tile_pool(name="ps", bufs=3, space="PSUM"))
        psb = ctx.enter_context(tc.tile_pool(name="psb", bufs=2, space="PSUM"))
        pso = ctx.enter_context(tc.tile_pool(name="pso", bufs=2, space="PSUM"))
        dram = ctx.enter_context(tc.tile_pool(name="dram", bufs=1,
                                              space="DRAM"))

        cp = const.tile([128, NCOL], F32, tag="cp", name="cp")
        nc.sync.dma_start(cp[:], cp_d.ap())

        def C(name, rows):
            c0, w = _cols[name]
            return cp[0:rows, c0:c0 + w]

        ident = C("ident", SEQ)
        id1 = cp[0:1, _cols["ident"][0]:_cols["ident"][0] + 1]

        # persistent standardized-activation tiles with a ones column 16
        # (transposing them yields the ones-augmented [17,123] layout)
        kstd = pers.tile([SEQ, D + 1], F32, tag="kstd", name="kstd")
        xstd = pers.tile([SEQ, D + 1], F32, tag="xstd", name="xstd")
        l1std = pers.tile([SEQ, D + 1], F32, tag="l1std", name="l1std")
        enc_s = pers.tile([SEQ, D + 1], F32, tag="encs", name="enc_s")
        for t_ in (kstd, xstd, l1std, enc_s):
            nc.vector.memset(t_[:, D:D + 1], 1.0)
        epsv = pers.tile([SEQ, 1], F32, tag="epsv", name="epsv")
        nc.vector.memset(epsv[:], EPS)
        onev = pers.tile([1, 1], F32, tag="onev", name="onev")
        nc.vector.memset(onev[:], 1.0)

        # x0 = 4*xq + POS ; xk0 = 4*xk + POS
        x = pers.tile([SEQ, D], F32, tag="x0", name="x0")
        nc.vector.scalar_tensor_tensor(x[:], C("xq", SEQ), SCL, C("pos", SEQ),
                                       op0=ALU.mult, op1=ALU.add)
        xk0 = pers.tile([SEQ, D], F32, tag="xk0", name="xk0")
        nc.vector.scalar_tensor_tensor(xk0[:], C("xk", SEQ), SCL,
                                       C("pos", SEQ), op0=ALU.mult,
                                       op1=ALU.add)

        # key-stream standardization (constant across layers: LN g/b folded)
        _ln_block(nc, sb, st, xk0[:], kstd[:, 0:D], "k", epsv[:])
        ksT_ps = ps.tile([D + 1, SEQ], F32, tag="ps16", name="ksT_ps")
        nc.tensor.transpose(ksT_ps[:], kstd[:], ident)
        knT = sb.tile([D + 1, SEQ], F32, tag="knT", bufs=1, name="knT")
        nc.scalar.copy(knT[:], ksT_ps[:])

        attn_s = None
        fco_s = None
        fch_keep = None
        for l in range(L):
            # ---- off critical path: K-side projections for this layer ----
            kT_ps = ps.tile([D, SEQ], F32, tag="ps16", name=f"kT_ps{l}")
            nc.tensor.matmul(kT_ps[:], C(f"wkb{l}", D + 1), knT[:],
                             start=True, stop=True)
            kT_s = sb.tile([D, SEQ], F32, tag="kT", name=f"kT_s{l}")
            nc.vector.tensor_copy(kT_s[:], kT_ps[:])
            m_ps = ps.tile([D + 1, SEQ], F32, tag="ps16", name=f"m_ps{l}")
            nc.tensor.matmul(m_ps[:], C(f"wqbT{l}", D), kT_s[:],
                             start=True, stop=True)
            m_s = sb.tile([D + 1, SEQ], F32, tag="m", name=f"m_s{l}")
            nc.vector.tensor_copy(m_s[:], m_ps[:])

            # vT with ones row (unit col in wvb selects knT ones row)
            vT_ps = ps.tile([D + 1, SEQ], F32, tag="ps16", name=f"vT_ps{l}")
            nc.tensor.matmul(vT_ps[:], C(f"wvb{l}", D + 1), knT[:],
                             start=True, stop=True)
            vT_s = sb.tile([D + 1, SEQ], F32, tag="vT", name=f"vT_s{l}")
            nc.vector.tensor_copy(vT_s[:], vT_ps[:])
            vo_ps = pso.tile([SEQ, D], F32, tag="pso", name=f"vo_ps{l}")
            nc.tensor.matmul(vo_ps[:], vT_s[:], C(f"wob{l}", D + 1),
                             start=True, stop=True)
            vo_s = sb.tile([SEQ, D], F32, tag="vo", name=f"vo_s{l}")
            nc.vector.tensor_copy(vo_s[:], vo_ps[:])

            # ---- critical path: Q-side LN -> scores -> softmax -> out ----
            _ln_block(nc, sb, st, x[:], xstd[:, 0:D], f"q{l}", epsv[:])
            xsT_ps = ps.tile([D + 1, SEQ], F32, tag="ps16", name=f"xsT_ps{l}")
            nc.tensor.transpose(xsT_ps[:], xstd[:], ident)
            xsT_s = sb.tile([D + 1, SEQ], F32, tag="xsT", name=f"xsT_s{l}")
            nc.scalar.copy(xsT_s[:], xsT_ps[:])
            sc_ps = psb.tile([SEQ, SEQ], F32, tag="big", name=f"sc_ps{l}")
            nc.tensor.matmul(sc_ps[:], xsT_s[:], m_s[:], start=True,
                             stop=True)
            # softmax without max-subtraction (scores are O(1) here; softmax
            # is shift-invariant so this matches the reference numerically)
            rowsum = st.tile([SEQ, 1], F32, tag="rsum", name=f"rowsum{l}")
            e_s = sb.tile([SEQ, SEQ], F32, tag="e", name=f"e_s{l}")
            nc.scalar.activation(e_s[:], sc_ps[:], AF.Exp,
                                 accum_out=rowsum[:])
            rinv = st.tile([SEQ, 1], F32, tag="rinv", name=f"rinv{l}")
            nc.vector.reciprocal(rinv[:], rowsum[:])
            attn_s = sb.tile([SEQ, SEQ], F32, tag="attn", name=f"attn_s{l}")
            nc.vector.tensor_scalar_mul(attn_s[:], e_s[:], rinv[:])
            aT_ps = psb.tile([SEQ, SEQ], F32, tag="big", name=f"aT_ps{l}")
            nc.tensor.transpose(aT_ps[:], attn_s[:], ident)
            aT_s = sb.tile([SEQ, SEQ], F32, tag="aT", name=f"aT_s{l}")
            nc.scalar.copy(aT_s[:], aT_ps[:])
            o_ps = pso.tile([SEQ, D], F32, tag="pso", name=f"o_ps{l}")
            nc.tensor.matmul(o_ps[:], aT_s[:], vo_s[:], start=True, stop=True)
            x2 = sb.tile([SEQ, D], F32, tag="x", bufs=3, name=f"x2_{l}")
            nc.vector.tensor_add(x2[:], o_ps[:], x[:])

            # ---- FFN ----
            _ln_block(nc, sb, st, x2[:], l1std[:, 0:D], f"f{l}", epsv[:])
            l1T_ps = ps.tile([D + 1, SEQ], F32, tag="ps16", name=f"l1T_ps{l}")
            nc.tensor.transpose(l1T_ps[:], l1std[:], ident)
            l1T_s = sb.tile([D + 1, SEQ], F32, tag="l1T", name=f"l1T_s{l}")
            nc.scalar.copy(l1T_s[:], l1T_ps[:])
            hT_ps = pso.tile([FF + 1, SEQ], F32, tag="pso", name=f"hT_ps{l}")
            nc.tensor.matmul(hT_ps[:], C(f"fc1b{l}", D + 1), l1T_s[:],
                             start=True, stop=True)
            hT_s = sb.tile([FF + 1, SEQ], F32, tag="hT", name=f"hT_s{l}")
            nc.scalar.activation(hT_s[:], hT_ps[:], AF.Relu)
            fco_ps = pso.tile([SEQ, D], F32, tag="pso", name=f"fco_ps{l}")
            nc.tensor.matmul(fco_ps[:], hT_s[:], C(f"fc2b{l}", FF + 1),
                             start=True, stop=True)
            if l == L - 1:
                fco_s = sb.tile([SEQ, D], F32, tag="fcos", bufs=1,
                                name="fco_s")
                nc.vector.tensor_copy(fco_s[:], fco_ps[:])
                fch_keep = hT_s
            x3 = sb.tile([SEQ, D], F32, tag="x", bufs=3, name=f"x3_{l}")
            nc.vector.tensor_add(x3[:], fco_ps[:], x2[:])
            x = x3

        # ---- final LN with real gain/bias, encoder output ----
        xfstd = sb.tile([SEQ, D], F32, tag="xfstd", bufs=1, name="xfstd")
        _ln_block(nc, sb, st, x[:], xfstd[:], "fin", epsv[:])
        enc_g = sb.tile([SEQ, D], F32, tag="encg", bufs=1, name="enc_g")
        nc.vector.tensor_mul(enc_g[:], xfstd[:], C("lnfg", SEQ))
        nc.vector.tensor_add(enc_s[:, 0:D], enc_g[:], C("lnfb", SEQ))

        # encoder outputs -> DRAM
        nc.sync.dma_start(attn_d.ap(), attn_s[:])
        nc.sync.dma_start(fch_d.ap(), fch_keep[0:FF, :])
        nc.sync.dma_start(fco_d.ap(), fco_s[:])
        nc.sync.dma_start(enc_d.ap(), enc_s[:, 0:D])

        # ---- conv branch on this core's encoder output ----
        encT_ps = ps.tile([D + 1, SEQ], F32, tag="ps16", name="encT_ps")
        nc.tensor.transpose(encT_ps[:], enc_s[:], ident)
        # zero-padded to width SEQ+9 so every conv tap covers the full
        # [123,30] PSUM region (well-formed accumulation group); padded
        # columns contribute zero to the (unused) tail windows.
        encT_s = sb.tile([D + 1, SEQ + NTAP - 1], F32, tag="encT", bufs=1,
                         name="encT_s")
        nc.vector.memset(encT_s[:, SEQ:], 0.0)
        nc.scalar.copy(encT_s[:, 0:SEQ], encT_ps[:])
        cv_ps = psb.tile([SEQ, 3 * OC], F32, tag="big", name="cv_ps")
        for t in range(NTAP):
            nc.tensor.matmul(cv_ps[:], encT_s[:, t:t + SEQ],
                             C(f"cw{t}", D + 1), start=(t == 0),
                             stop=(t == NTAP - 1))
        cvr_s = sb.tile([SEQ, 3 * OC], F32, tag="cvr", bufs=1, name="cvr_s")
        nc.scalar.activation(cvr_s[:], cv_ps[:], AF.Relu)
        tmaxs = []
        for j, k in enumerate(KS):
            cvT_ps = psb.tile([OC, SEQ], F32, tag="big", name=f"cvT_ps{j}")
            nc.tensor.transpose(cvT_ps[:],
                                cvr_s[:, j * OC:(j + 1) * OC], ident)
            tj = st.tile([OC, 1], F32, tag=f"tmax{j}", name=f"tmax{j}")
            nc.vector.reduce_max(tj[:], cvT_ps[:, 0:SEQ - k + 1], axis=AX.X)
            tmaxs.append(tj)
        # branch fc1: ft = relu(sum_j t_j @ fc1w_j + fc1_b)
        ft_ps = pso.tile([1, 30], F32, tag="pso", name="ft_ps")
        for j in range(3):
            nc.tensor.matmul(ft_ps[:], tmaxs[j][:], C(f"fc1w{j}", OC),
                             start=(j == 0), stop=False)
        nc.tensor.matmul(ft_ps[:], onev[:], C("fc1bb", 1), start=False,
                         stop=True)
        fb_s = sb.tile([1, 30], F32, tag="fb", bufs=1, name="fb_s")
        nc.scalar.activation(fb_s[:], ft_ps[:], AF.Relu)
        fbT_ps = pso.tile([30, 1], F32, tag="pso", name="fbT_ps")
        nc.tensor.transpose(fbT_ps[:], fb_s[:], id1)
        hv_s = sb.tile([30, 1], F32, tag="hv", bufs=1, name="hv_s")
        nc.scalar.copy(hv_s[:], fbT_ps[:])
        feat_ps = pso.tile([1, OC], F32, tag="pso", name="feat_ps")
        nc.tensor.matmul(feat_ps[:], hv_s[:], C("fc2w", 30), start=True,
                         stop=False)
        nc.tensor.matmul(feat_ps[:], onev[:], C("fc2bb", 1), start=False,
                         stop=True)
        feat_s = sb.tile([1, OC], F32, tag="feat", bufs=1, name="feat_s")
        nc.vector.tensor_copy(feat_s[:], feat_ps[:])

        # ---- AllGather the 4 branch features, then the shared head ----
        feat_dr = dram.tile([1, OC], F32, tag="featd", name="feat_dr")
        gath_dr = dram.tile([1, NB * OC], F32, tag="gathd", name="gath_dr")
        nc.sync.dma_start(feat_dr[:], feat_s[:])
        nc.gpsimd.collective_compute(
            "AllGather", ALU.bypass,
            replica_groups=[[0, 1, 2, 3], [4, 5, 6, 7]],
            ins=[feat_dr[:].opt()], outs=[gath_dr[:].opt()])
        z_s = sb.tile([1, NB * OC], F32, tag="z", bufs=1, name="z_s")
        nc.sync.dma_start(z_s[:], gath_dr[:])
        zT_ps = pso.tile([NB * OC, 1], F32, tag="pso", name="zT_ps")
        nc.tensor.transpose(zT_ps[:], z_s[:], id1)
        zv_s = sb.tile([NB * OC, 1], F32, tag="zv", bufs=1, name="zv_s")
        nc.scalar.copy(zv_s[:], zT_ps[:])
        s1_ps = pso.tile([1, 40], F32, tag="pso", name="s1_ps")
        nc.tensor.matmul(s1_ps[:], zv_s[:], C("w1", 40), start=True,
                         stop=False)
        nc.tensor.matmul(s1_ps[:], onev[:], C("w1bb", 1), start=False,
                         stop=True)
        # sigmoid(u) = 1 / (1 + exp(-u))
        e1_s = sb.tile([1, 40], F32, tag="e1", bufs=1, name="e1_s")
        nc.scalar.activation(e1_s[:], s1_ps[:], AF.Exp, scale=-1.0)
        p1_s = sb.tile([1, 40], F32, tag="p1", bufs=1, name="p1_s")
        nc.vector.tensor_scalar_add(p1_s[:], e1_s[:], 1.0)
        sg1_s = sb.tile([1, 40], F32, tag="sg1", bufs=1, name="sg1_s")
        nc.vector.reciprocal(sg1_s[:], p1_s[:])
        s1T_ps = pso.tile([40, 1], F32, tag="pso", name="s1T_ps")
        nc.tensor.transpose(s1T_ps[:], sg1_s[:], id1)
        sv_s = sb.tile([40, 1], F32, tag="sv", bufs=1, name="sv_s")
        nc.scalar.copy(sv_s[:], s1T_ps[:])
        o2_ps = pso.tile([1, 2], F32, tag="pso", name="o2_ps")
        nc.tensor.matmul(o2_ps[:], sv_s[:], C("w2", 40), start=True,
                         stop=False)
        nc.tensor.matmul(o2_ps[:], onev[:], C("w2bb", 1), start=False,
                         stop=True)
        e2_s = sb.tile([1, 2], F32, tag="e2", bufs=1, name="e2_s")
        nc.scalar.activation(e2_s[:], o2_ps[:], AF.Exp, scale=-1.0)
        p2_s = sb.tile([1, 2], F32, tag="p2", bufs=1, name="p2_s")
        nc.vector.tensor_scalar_add(p2_s[:], e2_s[:], 1.0)
        out_s = sb.tile([1, 2], F32, tag="outs", bufs=1, name="out_s")
        nc.vector.reciprocal(out_s[:], p2_s[:])
        nc.sync.dma_start(head_d.ap(), out_s[:])

    nc.compile()
    return nc


_NC_CACHE: list = []


def _get_nc():
    if not _NC_CACHE:
        _NC_CACHE.append(_build_nc())
    return _NC_CACHE[0]


# ------------------------------------------------------------------- driver --
def _np_tree(obj):
    if isinstance(obj, dict):
        return {k: _np_tree(v) for k, v in obj.items()}
    return np.asarray(obj)


def _prep(x, params):
    """Host-side input prep: build the 8 per-core constant packs."""
    x = np.asarray(x)
    p = _np_tree(params)

    wavA = x[0, 0, 0]
    wavB = x[0, 0, -1]
    eeg = x[0, 0, 1:-1, TD:WIN - 1]                  # (16, SEQ)
    idx = (np.arange(TD)[:, None] + 1) + np.arange(SEQ)[None, :]
    wA = np.ascontiguousarray(wavA[idx].T)           # (SEQ, 16)
    wB = np.ascontiguousarray(wavB[idx].T)
    eeg_t = np.ascontiguousarray(eeg.T)              # (SEQ, 16)

    def conv_params(i):
        return dict(w1=p["conv1_w"][i], b1=p["conv1_b"][i],
                    w2=p["conv2_w"][i], b2=p["conv2_b"][i],
                    w3=p["conv3_w"][i], b3=p["conv3_b"][i],
                    fc1_w=p["fc1_w"][i], fc1_b=p["fc1_b"][i],
                    fc2_w=p["fc2_w"][i], fc2_b=p["fc2_b"][i])

    head = dict(w1=p["out_w1"], b1=p["out_b1"], w2=p["out_w2"], b2=p["out_b2"])

    # core -> (encoder params, xq, xk, conv branch index)
    plan = [
        (p["a2e"], wA, eeg_t, 0),     # wavA_t  / branch 0
        (p["e2a"], eeg_t, wA, 1),     # eegA    / branch 1
        (p["e2a2"], eeg_t, wB, 2),    # eegB    / branch 2
        (p["a2e2"], wB, eeg_t, 3),    # wavB_t  / branch 3
    ]
    packs = [_pack_core(xq, xk, encp, conv_params(bi), head)
             for encp, xq, xk, bi in plan]
    in_maps = [{"cpack": packs[c % 4]} for c in range(8)]
    return in_maps, wA


def kernel(x, params):
    in_maps, wA = _prep(x, params)
    nc = _get_nc()
    res = run_bass_kernel_spmd(nc, in_maps, list(range(8))).results

    out = res[0]["head"].astype(np.float32)
    aeA = res[0]["attn"]
    eaA = res[1]["attn"]
    eaB = res[2]["attn"]
    aeB = res[3]["attn"]
    wavA_before = wA[None].astype(np.float32)
    wavA_after = res[0]["enc"][None]
    ae_fcA = np.ascontiguousarray(res[0]["fch"].T)
    ae_fc_lastA = res[0]["fco"]
    return (out, aeA, (wavA_before, wavA_after), ae_fcA, eaA, aeB, eaB,
            ae_fc_lastA)


# revision 18
# speedup vs baseline: 1.8948x; 1.8948x over previous
"""Trainium2 Bass kernel for the MulT-style cross-modal CNN/transformer.

Strategy (SPMD over 8 NeuronCores):
  - The model contains 4 independent 5-layer cross-modal encoders followed by
    4 independent conv branches and a tiny shared head. Core c (c in 0..3)
    runs encoder c and conv branch c on its own encoder output; cores 4..7
    mirror cores 0..3 (same program, same data -> harmless redundancy).
  - Branch features (1,10) are AllGather'd within {0,1,2,3} (and {4,5,6,7}),
    and the 2-layer sigmoid head is computed redundantly on every core.
  - Host-side prep folds all LayerNorm gains/biases into the adjacent weight
    matrices, folds all biases into ones-row augmented matmuls, pre-applies
    the attention output projection to V (vo = v @ Wo + bo), and packs every
    constant this core needs into ONE [128, NCOL] f32 tensor -> one DMA.
  - On-device transcendentals use only the `natural_log_exp_and_others`
    activation table (copy/relu/ln/exp): rsqrt(v) = exp(-0.5*ln(v+eps)),
    sigmoid(x) = 1/(1+exp(-x)) -> exactly one ACT table load per run.
  - All SBUF/PSUM access patterns start at partition 0 (HW requires
    quadrant-aligned partition offsets). Ones-augmentation rows are produced
    by transposing a ones-column or by an extra unit column in the
    stationary weights; small-vector bias adds use extra accumulating
    matmuls against a [1,1] ones tile.
"""

import numpy as np
from contextlib import ExitStack

import concourse.bass as bass
import concourse.tile as tile
from concourse import bacc, mybir
from concourse.bass_utils import run_bass_kernel_spmd

# The act-table chooser statically maps each activation function to the
# first table set containing it (Exp -> exp_and_others, Ln -> natural_log),
# which thrashes ACT_TABLE_LOAD (~1.3us each) on every Ln/Exp alternation.
# Every function this kernel uses (copy/relu/ln/exp/square/identity) lives
# in the natural_log set, so mask out all other sets (preserving dict order,
# hence act_func_set_id indices) to get exactly one table load per run.
_orig_gat = bacc.get_activation_tables


def _gat_single_set(arch):
    out = {}
    for name, fns in _orig_gat(arch).items():
        out[name] = fns if name.startswith("natural_log") else set()
    return out


bacc.get_activation_tables = _gat_single_set

F32 = mybir.dt.float32
AF = mybir.ActivationFunctionType
ALU = mybir.AluOpType
AX = mybir.AxisListType

SEQ, D, L, FF, OC, NB = 123, 16, 5, 64, 10, 4
WIN, TD = 140, 16
KS = (8, 9, 10)            # conv kernel sizes
NTAP = max(KS)             # 10 accumulation taps
EPS = 1e-5
SCL = float(D) ** 0.5      # 4.0
ISC = float(D) ** -0.5     # 0.25

# ------------------------------------------------------------------ layout --
# One [128, NCOL] f32 constants tensor per core. Column allocator:
_cols: dict[str, tuple[int, int]] = {}
_ncol = 0


def _alloc(name: str, cols: int) -> None:
    global _ncol
    _cols[name] = (_ncol, cols)
    _ncol += cols


_alloc("ident", SEQ)           # [123,123] identity (rows 0..122)
_alloc("xq", D)                # [123,16] raw query-stream input
_alloc("xk", D)                # [123,16] raw key-stream input
_alloc("pos", D)               # [123,16] positional embedding
_alloc("lnfg", D)              # [123,16] final-LN gain, broadcast
_alloc("lnfb", D)              # [123,16] final-LN bias, broadcast
for l in range(L):
    _alloc(f"wqbT{l}", D + 1)  # [16,17]  (Wq' ; bq')^T
    _alloc(f"wkb{l}", D)       # [17,16]
    _alloc(f"wvb{l}", D + 1)   # [17,17]  (Wv' ; bv') + unit col -> ones row
    _alloc(f"wob{l}", D)       # [17,16]
    _alloc(f"fc1b{l}", FF + 1)  # [17,65] (fc1';b1') + unit col -> ones row
    _alloc(f"fc2b{l}", D)      # [65,16]
for t in range(NTAP):
    _alloc(f"cw{t}", 3 * OC)   # [17,30] conv tap t, all 3 convs (+bias on t=0)
for j in range(3):
    _alloc(f"fc1w{j}", 30)     # [10,30] branch fc1 weight rows j*10..j*10+9
_alloc("fc1bb", 30)            # [1,30] branch fc1 bias
_alloc("fc2w", OC)             # [30,10]
_alloc("fc2bb", OC)            # [1,10]
_alloc("w1", 40)               # [40,40]
_alloc("w1bb", 40)             # [1,40]
_alloc("w2", 2)                # [40,2]
_alloc("w2bb", 2)              # [1,2]
NCOL = _ncol

E16 = np.zeros((17, 1))
E16[16, 0] = 1.0


def _pos_embed() -> np.ndarray:
    half = D // 2
    freqs = np.exp(np.arange(half) * (-np.log(10000.0) / (half - 1)))
    ang = np.arange(SEQ)[:, None] * freqs[None, :]
    return np.concatenate([np.sin(ang), np.cos(ang)], axis=1).astype(np.float32)


def _pack_core(xq: np.ndarray, xk: np.ndarray, enc: dict, conv: dict,
               head: dict) -> np.ndarray:
    """Build the [128, NCOL] constants tensor for one core (float64 folding)."""
    cp = np.zeros((128, NCOL), np.float64)

    def put(name: str, arr: np.ndarray) -> None:
        c0, w = _cols[name]
        assert arr.shape[1] == w, (name, arr.shape, w)
        cp[: arr.shape[0], c0:c0 + w] = arr

    put("ident", np.eye(SEQ))
    put("xq", xq.astype(np.float64))
    put("xk", xk.astype(np.float64))
    put("pos", _pos_embed().astype(np.float64))
    put("lnfg", np.broadcast_to(enc["lnf_g"], (SEQ, D)))
    put("lnfb", np.broadcast_to(enc["lnf_b"], (SEQ, D)))
    for l in range(L):
        g0, b0 = enc["ln0_g"][l], enc["ln0_b"][l]
        gk, bk_ = enc["ln0k_g"][l], enc["ln0k_b"][l]
        g1, b1 = enc["ln1_g"][l], enc["ln1_b"][l]
        wq, wk, wv, wo = enc["Wq"][l], enc["Wk"][l], enc["Wv"][l], enc["Wo"][l]
        wqb = np.vstack([g0[:, None] * wq * ISC,
                         (b0 @ wq + enc["bq"][l])[None] * ISC])       # [17,16]
        put(f"wqbT{l}", wqb.T)                                        # [16,17]
        put(f"wkb{l}", np.vstack([gk[:, None] * wk,
                                  (bk_ @ wk + enc["bk"][l])[None]]))
        wvb = np.vstack([gk[:, None] * wv, (bk_ @ wv + enc["bv"][l])[None]])
        put(f"wvb{l}", np.hstack([wvb, E16]))                         # [17,17]
        put(f"wob{l}", np.vstack([wo, enc["bo"][l][None]]))
        f1 = np.vstack([g1[:, None] * enc["fc1_w"][l],
                        (b1 @ enc["fc1_w"][l] + enc["fc1_b"][l])[None]])
        put(f"fc1b{l}", np.hstack([f1, E16]))                         # [17,65]
        put(f"fc2b{l}", np.vstack([enc["fc2_w"][l], enc["fc2_b"][l][None]]))
    # conv taps: cw_t[ic, j*10+oc] = w_j[oc, ic, t] (0 when t >= k_j);
    # bias row 16 only on tap 0 (multiplied by the enc ones-row).
    for t in range(NTAP):
        cw = np.zeros((17, 3 * OC), np.float64)
        for j, k in enumerate(KS):
            if t < k:
                cw[:D, j * OC:(j + 1) * OC] = conv[f"w{j+1}"][:, :, t].T
        if t == 0:
            cw[D, :] = np.concatenate([conv["b1"], conv["b2"], conv["b3"]])
        put(f"cw{t}", cw)
    for j in range(3):
        put(f"fc1w{j}", conv["fc1_w"][j * OC:(j + 1) * OC])
    put("fc1bb", conv["fc1_b"][None])
    put("fc2w", conv["fc2_w"])
    put("fc2bb", conv["fc2_b"][None])
    put("w1", head["w1"])
    put("w1bb", head["b1"][None])
    put("w2", head["w2"])
    put("w2bb", head["b2"][None])
    return np.ascontiguousarray(cp, np.float32)


# ------------------------------------------------------------- device build --
def _ln_block(nc, sb, st, x_ap, sum_ap, out_ap, tag, eps_ap):
    """LayerNorm standardize: out = (x - mean(x)) * rsqrt(var(x) + eps).
    `sum_ap` is the precomputed row-sum of x (captured for free via
    accum_out on the op that materialized x). Gains/biases are folded into
    downstream weights by the host."""
    negmean = st.tile([SEQ, 1], F32, tag=f"nm{tag}", name=f"nm{tag}")
    nc.vector.tensor_scalar_mul(negmean[:], sum_ap, -1.0 / D)
    ssq = st.tile([SEQ, 1], F32, tag=f"sq{tag}", name=f"sq{tag}")
    scr2 = sb.tile([SEQ, D], F32, tag="scr", bufs=2, name=f"scr2{tag}")
    # DVE: out = (x - mean) * x, accum = sum((x-m)x) = sum((x-m)^2)
    nc.vector.scalar_tensor_tensor(scr2[:], x_ap, negmean[:], x_ap,
                                   op0=ALU.add, op1=ALU.mult,
                                   accum_out=ssq[:])
    lnv = st.tile([SEQ, 1], F32, tag=f"lv{tag}", name=f"lv{tag}")
    nc.scalar.activation(lnv[:], ssq[:], AF.Ln, scale=1.0 / D, bias=eps_ap)
    rstd = st.tile([SEQ, 1], F32, tag=f"rs{tag}", name=f"rs{tag}")
    nc.scalar.activation(rstd[:], lnv[:], AF.Exp, scale=-0.5)
    # DVE: out = (x + negmean) * rstd
    nc.vector.tensor_scalar(out_ap, x_ap, negmean[:], rstd[:],
                            op0=ALU.add, op1=ALU.mult)


def _build_nc():
    nc = bacc.Bacc("TRN2", target_bir_lowering=False, debug=False,
                   num_devices=8)
    cp_d = nc.dram_tensor("cpack", [128, NCOL], F32, kind="ExternalInput")
    attn_d = nc.dram_tensor("attn", [SEQ, SEQ], F32, kind="ExternalOutput")
    fch_d = nc.dram_tensor("fch", [FF, SEQ], F32, kind="ExternalOutput")
    fco_d = nc.dram_tensor("fco", [SEQ, D], F32, kind="ExternalOutput")
    enc_d = nc.dram_tensor("enc", [SEQ, D], F32, kind="ExternalOutput")
    feat_d = nc.dram_tensor("feat", [1, OC], F32, kind="ExternalOutput")

    with tile.TileContext(nc, trace_sim=False) as tc, ExitStack() as ctx:
        const = ctx.enter_context(tc.tile_pool(name="const", bufs=1))
        pers = ctx.enter_context(tc.tile_pool(name="pers", bufs=1))
        sb = ctx.enter_context(tc.tile_pool(name="sb", bufs=2))
        st = ctx.enter_context(tc.tile_pool(name="st", bufs=2))
        ps = ctx.enter_context(tc.tile_pool(name="ps", bufs=3, space="PSUM"))
        psb = ctx.enter_context(tc.tile_pool(name="psb", bufs=2, space="PSUM"))
        pso = ctx.enter_context(tc.tile_pool(name="pso", bufs=2, space="PSUM"))

        cp = const.tile([128, NCOL], F32, tag="cp", name="cp")
        nc.sync.dma_start(cp[:], cp_d.ap())

        def C(name, rows):
            c0, w = _cols[name]
            return cp[0:rows, c0:c0 + w]

        ident = C("ident", SEQ)
        id1 = cp[0:1, _cols["ident"][0]:_cols["ident"][0] + 1]

        # persistent standardized-activation tiles with a ones column 16
        # (transposing them yields the ones-augmented [17,123] layout)
        kstd = pers.tile([SEQ, D + 1], F32, tag="kstd", name="kstd")
        xstd = pers.tile([SEQ, D + 1], F32, tag="xstd", name="xstd")
        l1std = pers.tile([SEQ, D + 1], F32, tag="l1std", name="l1std")
        enc_s = pers.tile([SEQ, D + 1], F32, tag="encs", name="enc_s")
        for t_ in (kstd, xstd, l1std, enc_s):
            nc.vector.memset(t_[:, D:D + 1], 1.0)
        epsv = pers.tile([SEQ, 1], F32, tag="epsv", name="epsv")
        nc.vector.memset(epsv[:], EPS)
        onev = pers.tile([1, 1], F32, tag="onev", name="onev")
        nc.vector.memset(onev[:], 1.0)

        # x0 = 4*xq + POS ; xk0 = 4*xk + POS (row-sums captured for the LNs)
        x = pers.tile([SEQ, D], F32, tag="x0", name="x0")
        xsum = st.tile([SEQ, 1], F32, tag="xsum", bufs=3, name="xsum0")
        nc.vector.scalar_tensor_tensor(x[:], C("xq", SEQ), SCL, C("pos", SEQ),
                                       op0=ALU.mult, op1=ALU.add,
                                       accum_out=xsum[:])
        xk0 = pers.tile([SEQ, D], F32, tag="xk0", name="xk0")
        ksum = st.tile([SEQ, 1], F32, tag="ksum", bufs=1, name="ksum")
        nc.vector.scalar_tensor_tensor(xk0[:], C("xk", SEQ), SCL,
                                       C("pos", SEQ), op0=ALU.mult,
                                       op1=ALU.add, accum_out=ksum[:])

        # key-stream standardization (constant across layers: LN g/b folded)
        _ln_block(nc, sb, st, xk0[:], ksum[:], kstd[:, 0:D], "k", epsv[:])
        ksT_ps = ps.tile([D + 1, SEQ], F32, tag="ps16", name="ksT_ps")
        nc.tensor.transpose(ksT_ps[:], kstd[:], ident)
        knT = sb.tile([D + 1, SEQ], F32, tag="knT", bufs=1, name="knT")
        nc.scalar.copy(knT[:], ksT_ps[:])

        attn_s = None
        fco_s = None
        fch_keep = None
        for l in range(L):
            # ---- off critical path: K-side projections for this layer ----
            kT_ps = ps.tile([D, SEQ], F32, tag="ps16", name=f"kT_ps{l}")
            nc.tensor.matmul(kT_ps[:], C(f"wkb{l}", D + 1), knT[:],
                             start=True, stop=True)
            kT_s = sb.tile([D, SEQ], F32, tag="kT", name=f"kT_s{l}")
            nc.vector.tensor_copy(kT_s[:], kT_ps[:])
            m_ps = ps.tile([D + 1, SEQ], F32, tag="ps16", name=f"m_ps{l}")
            nc.tensor.matmul(m_ps[:], C(f"wqbT{l}", D), kT_s[:],
                             start=True, stop=True)
            m_s = sb.tile([D + 1, SEQ], F32, tag="m", name=f"m_s{l}")
            nc.vector.tensor_copy(m_s[:], m_ps[:])

            # vT with ones row (unit col in wvb selects knT ones row)
            vT_ps = ps.tile([D + 1, SEQ], F32, tag="ps16", name=f"vT_ps{l}")
            nc.tensor.matmul(vT_ps[:], C(f"wvb{l}", D + 1), knT[:],
                             start=True, stop=True)
            vT_s = sb.tile([D + 1, SEQ], F32, tag="vT", name=f"vT_s{l}")
            nc.vector.tensor_copy(vT_s[:], vT_ps[:])
            vo_ps = pso.tile([SEQ, D], F32, tag="pso", name=f"vo_ps{l}")
            nc.tensor.matmul(vo_ps[:], vT_s[:], C(f"wob{l}", D + 1),
                             start=True, stop=True)
            vo_s = sb.tile([SEQ, D], F32, tag="vo", name=f"vo_s{l}")
            nc.vector.tensor_copy(vo_s[:], vo_ps[:])

            # ---- critical path: Q-side LN -> scores -> softmax -> out ----
            _ln_block(nc, sb, st, x[:], xsum[:], xstd[:, 0:D], f"q{l}",
                      epsv[:])
            xsT_ps = ps.tile([D + 1, SEQ], F32, tag="ps16", name=f"xsT_ps{l}")
            nc.tensor.transpose(xsT_ps[:], xstd[:], ident)
            xsT_s = sb.tile([D + 1, SEQ], F32, tag="xsT", name=f"xsT_s{l}")
            nc.scalar.copy(xsT_s[:], xsT_ps[:])
            sc_ps = psb.tile([SEQ, SEQ], F32, tag="big", name=f"sc_ps{l}")
            nc.tensor.matmul(sc_ps[:], xsT_s[:], m_s[:], start=True,
                             stop=True)
            # softmax without max-subtraction (scores are O(1) here; softmax
            # is shift-invariant so this matches the reference numerically)
            rowsum = st.tile([SEQ, 1], F32, tag="rsum", name=f"rowsum{l}")
            e_s = sb.tile([SEQ, SEQ], F32, tag="e", name=f"e_s{l}")
            nc.scalar.activation(e_s[:], sc_ps[:], AF.Exp,
                                 accum_out=rowsum[:])
            rinv = st.tile([SEQ, 1], F32, tag="rinv", name=f"rinv{l}")
            nc.vector.reciprocal(rinv[:], rowsum[:])
            attn_s = sb.tile([SEQ, SEQ], F32, tag="attn", name=f"attn_s{l}")
            nc.vector.tensor_scalar_mul(attn_s[:], e_s[:], rinv[:])
            aT_ps = psb.tile([SEQ, SEQ], F32, tag="big", name=f"aT_ps{l}")
            nc.tensor.transpose(aT_ps[:], attn_s[:], ident)
            aT_s = sb.tile([SEQ, SEQ], F32, tag="aT", name=f"aT_s{l}")
            nc.scalar.copy(aT_s[:], aT_ps[:])
            o_ps = pso.tile([SEQ, D], F32, tag="pso", name=f"o_ps{l}")
            nc.tensor.matmul(o_ps[:], aT_s[:], vo_s[:], start=True, stop=True)
            x2 = sb.tile([SEQ, D], F32, tag="x", bufs=3, name=f"x2_{l}")
            x2sum = st.tile([SEQ, 1], F32, tag="xsum", bufs=3,
                            name=f"x2sum{l}")
            nc.vector.scalar_tensor_tensor(x2[:], o_ps[:], 1.0, x[:],
                                           op0=ALU.mult, op1=ALU.add,
                                           accum_out=x2sum[:])

            # ---- FFN ----
            _ln_block(nc, sb, st, x2[:], x2sum[:], l1std[:, 0:D], f"f{l}",
                      epsv[:])
            l1T_ps = ps.tile([D + 1, SEQ], F32, tag="ps16", name=f"l1T_ps{l}")
            nc.tensor.transpose(l1T_ps[:], l1std[:], ident)
            l1T_s = sb.tile([D + 1, SEQ], F32, tag="l1T", name=f"l1T_s{l}")
            nc.scalar.copy(l1T_s[:], l1T_ps[:])
            hT_ps = pso.tile([FF + 1, SEQ], F32, tag="pso", name=f"hT_ps{l}")
            nc.tensor.matmul(hT_ps[:], C(f"fc1b{l}", D + 1), l1T_s[:],
                             start=True, stop=True)
            hT_s = sb.tile([FF + 1, SEQ], F32, tag="hT", name=f"hT_s{l}")
            nc.scalar.activation(hT_s[:], hT_ps[:], AF.Relu)
            fco_ps = pso.tile([SEQ, D], F32, tag="pso", name=f"fco_ps{l}")
            nc.tensor.matmul(fco_ps[:], hT_s[:], C(f"fc2b{l}", FF + 1),
                             start=True, stop=True)
            if l == L - 1:
                fco_s = sb.tile([SEQ, D], F32, tag="fcos", bufs=1,
                                name="fco_s")
                nc.vector.tensor_copy(fco_s[:], fco_ps[:])
                fch_keep = hT_s
            x3 = sb.tile([SEQ, D], F32, tag="x", bufs=3, name=f"x3_{l}")
            xsum = st.tile([SEQ, 1], F32, tag="xsum", bufs=3,
                           name=f"x3sum{l}")
            nc.vector.scalar_tensor_tensor(x3[:], fco_ps[:], 1.0, x2[:],
                                           op0=ALU.mult, op1=ALU.add,
                                           accum_out=xsum[:])
            x = x3

        # ---- final LN with real gain/bias, encoder output ----
        xfstd = sb.tile([SEQ, D], F32, tag="xfstd", bufs=1, name="xfstd")
        _ln_block(nc, sb, st, x[:], xsum[:], xfstd[:], "fin", epsv[:])
        enc_g = sb.tile([SEQ, D], F32, tag="encg", bufs=1, name="enc_g")
        nc.vector.tensor_mul(enc_g[:], xfstd[:], C("lnfg", SEQ))
        nc.vector.tensor_add(enc_s[:, 0:D], enc_g[:], C("lnfb", SEQ))

        # encoder outputs -> DRAM
        nc.sync.dma_start(attn_d.ap(), attn_s[:])
        nc.sync.dma_start(fch_d.ap(), fch_keep[0:FF, :])
        nc.sync.dma_start(fco_d.ap(), fco_s[:])
        nc.sync.dma_start(enc_d.ap(), enc_s[:, 0:D])

        # ---- conv branch on this core's encoder output ----
        encT_ps = ps.tile([D + 1, SEQ], F32, tag="ps16", name="encT_ps")
        nc.tensor.transpose(encT_ps[:], enc_s[:], ident)
        # zero-padded to width SEQ+9 so every conv tap covers the full
        # [123,30] PSUM region (well-formed accumulation group); padded
        # columns contribute zero to the (unused) tail windows.
        encT_s = sb.tile([D + 1, SEQ + NTAP - 1], F32, tag="encT", bufs=1,
                         name="encT_s")
        nc.vector.memset(encT_s[:, SEQ:], 0.0)
        nc.scalar.copy(encT_s[:, 0:SEQ], encT_ps[:])
        cv_ps = psb.tile([SEQ, 3 * OC], F32, tag="big", name="cv_ps")
        for t in range(NTAP):
            nc.tensor.matmul(cv_ps[:], encT_s[:, t:t + SEQ],
                             C(f"cw{t}", D + 1), start=(t == 0),
                             stop=(t == NTAP - 1))
        cvr_s = sb.tile([SEQ, 3 * OC], F32, tag="cvr", bufs=1, name="cvr_s")
        nc.scalar.activation(cvr_s[:], cv_ps[:], AF.Relu)
        tmaxs = []
        for j, k in enumerate(KS):
            cvT_ps = psb.tile([OC, SEQ], F32, tag="big", name=f"cvT_ps{j}")
            nc.tensor.transpose(cvT_ps[:],
                                cvr_s[:, j * OC:(j + 1) * OC], ident)
            tj = st.tile([OC, 1], F32, tag=f"tmax{j}", name=f"tmax{j}")
            nc.vector.reduce_max(tj[:], cvT_ps[:, 0:SEQ - k + 1], axis=AX.X)
            tmaxs.append(tj)
        # branch fc1: ft = relu(sum_j t_j @ fc1w_j + fc1_b)
        ft_ps = pso.tile([1, 30], F32, tag="pso", name="ft_ps")
        for j in range(3):
            nc.tensor.matmul(ft_ps[:], tmaxs[j][:], C(f"fc1w{j}", OC),
                             start=(j == 0), stop=False)
        nc.tensor.matmul(ft_ps[:], onev[:], C("fc1bb", 1), start=False,
                         stop=True)
        fb_s = sb.tile([1, 30], F32, tag="fb", bufs=1, name="fb_s")
        nc.scalar.activation(fb_s[:], ft_ps[:], AF.Relu)
        fbT_ps = pso.tile([30, 1], F32, tag="pso", name="fbT_ps")
        nc.tensor.transpose(fbT_ps[:], fb_s[:], id1)
        hv_s = sb.tile([30, 1], F32, tag="hv", bufs=1, name="hv_s")
        nc.scalar.copy(hv_s[:], fbT_ps[:])
        feat_ps = pso.tile([1, OC], F32, tag="pso", name="feat_ps")
        nc.tensor.matmul(feat_ps[:], hv_s[:], C("fc2w", 30), start=True,
                         stop=False)
        nc.tensor.matmul(feat_ps[:], onev[:], C("fc2bb", 1), start=False,
                         stop=True)
        feat_s = sb.tile([1, OC], F32, tag="feat", bufs=1, name="feat_s")
        nc.vector.tensor_copy(feat_s[:], feat_ps[:])
        nc.sync.dma_start(feat_d.ap(), feat_s[:])

    nc.compile()
    return nc


_NC_CACHE: list = []


def _get_nc():
    if not _NC_CACHE:
        _NC_CACHE.append(_build_nc())
    return _NC_CACHE[0]


# ------------------------------------------------------------------- driver --
def _np_tree(obj):
    if isinstance(obj, dict):
        return {k: _np_tree(v) for k, v in obj.items()}
    return np.asarray(obj)


def _prep(x, params):
    """Host-side input prep: build the 8 per-core constant packs."""
    x = np.asarray(x)
    p = _np_tree(params)

    wavA = x[0, 0, 0]
    wavB = x[0, 0, -1]
    eeg = x[0, 0, 1:-1, TD:WIN - 1]                  # (16, SEQ)
    idx = (np.arange(TD)[:, None] + 1) + np.arange(SEQ)[None, :]
    wA = np.ascontiguousarray(wavA[idx].T)           # (SEQ, 16)
    wB = np.ascontiguousarray(wavB[idx].T)
    eeg_t = np.ascontiguousarray(eeg.T)              # (SEQ, 16)

    def conv_params(i):
        return dict(w1=p["conv1_w"][i], b1=p["conv1_b"][i],
                    w2=p["conv2_w"][i], b2=p["conv2_b"][i],
                    w3=p["conv3_w"][i], b3=p["conv3_b"][i],
                    fc1_w=p["fc1_w"][i], fc1_b=p["fc1_b"][i],
                    fc2_w=p["fc2_w"][i], fc2_b=p["fc2_b"][i])

    head = dict(w1=p["out_w1"], b1=p["out_b1"], w2=p["out_w2"], b2=p["out_b2"])

    # core -> (encoder params, xq, xk, conv branch index)
    plan = [
        (p["a2e"], wA, eeg_t, 0),     # wavA_t  / branch 0
        (p["e2a"], eeg_t, wA, 1),     # eegA    / branch 1
        (p["e2a2"], eeg_t, wB, 2),    # eegB    / branch 2
        (p["a2e2"], wB, eeg_t, 3),    # wavB_t  / branch 3
    ]
    packs = [_pack_core(xq, xk, encp, conv_params(bi), head)
             for encp, xq, xk, bi in plan]
    in_maps = [{"cpack": packs[c % 4]} for c in range(8)]
    return in_maps, wA


def _host_head(feats, p):
    """Tiny 2-layer sigmoid head (1.7 KFLOP) on the gathered features."""
    z = np.concatenate([f.astype(np.float64) for f in feats], axis=1)
    s1 = 1.0 / (1.0 + np.exp(-(z @ p["out_w1"].astype(np.float64)
                               + p["out_b1"].astype(np.float64))))
    o = 1.0 / (1.0 + np.exp(-(s1 @ p["out_w2"].astype(np.float64)
                              + p["out_b2"].astype(np.float64))))
    return o.astype(np.float32)


def kernel(x, params):
    in_maps, wA = _prep(x, params)
    nc = _get_nc()
    res = run_bass_kernel_spmd(nc, in_maps, list(range(8))).results

    p = _np_tree(params)
    out = _host_head([res[c]["feat"] for c in range(4)], p)
    aeA = res[0]["attn"]
    eaA = res[1]["attn"]
    eaB = res[2]["attn"]
    aeB = res[3]["attn"]
    wavA_before = wA[None].astype(np.float32)
    wavA_after = res[0]["enc"][None]
    ae_fcA = np.ascontiguousarray(res[0]["fch"].T)
    ae_fc_lastA = res[0]["fco"]
    return (out, aeA, (wavA_before, wavA_after), ae_fcA, eaA, aeB, eaB,
            ae_fc_lastA)


# revision 24
# speedup vs baseline: 1.9597x; 1.0342x over previous
"""Trainium2 Bass kernel for the MulT-style cross-modal CNN/transformer.

Strategy (SPMD over 8 NeuronCores):
  - The model contains 4 independent 5-layer cross-modal encoders followed by
    4 independent conv branches and a tiny shared head. Core c (c in 0..3)
    runs encoder c and conv branch c on its own encoder output; cores 4..7
    mirror cores 0..3 (same program, same data -> harmless redundancy).
  - Branch features (1,10) are per-core outputs; the 1.7-KFLOP sigmoid head
    runs on the host (collectives cost ~80us of latency for 160 bytes).
  - Host-side prep folds all LayerNorm gains/biases into the adjacent weight
    matrices, folds biases into ones-row augmented matmuls, pre-applies the
    attention output projection to V (vo = v @ Wo + bo), and packs every
    constant this core needs into ONE [128, NCOL] f32 tensor -> one DMA.
  - PE instruction count dominates (fixed ~400ns/matmul at these sizes), so
    the K-side projections of all 5 layers are batched into 32-partition-
    aligned block matmuls, the 3 convs into one 32-aligned block layout, and
    softmax normalization is folded into the residual add (1/rowsum scaling)
    so it leaves the critical path.
  - On-device transcendentals use only the `natural_log_exp_and_others`
    activation table (copy/relu/ln/exp): rsqrt(v) = exp(-0.5*ln(v+eps)) ->
    exactly one ACT table load per run.
  - All SBUF/PSUM access patterns start at partition offsets that are
    multiples of 32 (HW quadrant constraint).
"""

import numpy as np
from contextlib import ExitStack

import concourse.bass as bass
import concourse.tile as tile
from concourse import bacc, mybir
from concourse.bass_utils import run_bass_kernel_spmd

# The act-table chooser statically maps each activation function to the
# first table set containing it (Exp -> exp_and_others, Ln -> natural_log),
# which thrashes ACT_TABLE_LOAD (~1.3us each) on every Ln/Exp alternation.
# Every function this kernel uses (copy/relu/ln/exp) lives in the
# natural_log set, so mask all other sets (preserving dict order, hence
# act_func_set_id indices) to get exactly one table load per run.
_orig_gat = bacc.get_activation_tables


def _gat_single_set(arch):
    out = {}
    for name, fns in _orig_gat(arch).items():
        out[name] = fns if name.startswith("natural_log") else set()
    return out


bacc.get_activation_tables = _gat_single_set

F32 = mybir.dt.float32
AF = mybir.ActivationFunctionType
ALU = mybir.AluOpType
AX = mybir.AxisListType

SEQ, D, L, FF, OC, NB = 123, 16, 5, 64, 10, 4
WIN, TD = 140, 16
KS = (8, 9, 10)            # conv kernel sizes
NTAP = max(KS)             # 10 accumulation taps
EPS = 1e-5
SCL = float(D) ** 0.5      # 4.0
ISC = float(D) ** -0.5     # 0.25
# layer batches for the K-side block matmuls: (start layer, n layers)
BATCH = ((0, 3), (3, 2))

# ------------------------------------------------------------------ layout --
_cols: dict[str, tuple[int, int]] = {}
_ncol = 0


def _alloc(name: str, cols: int) -> None:
    global _ncol
    _cols[name] = (_ncol, cols)
    _ncol += cols


_alloc("ident", SEQ)           # [123,123] identity
_alloc("xq", D)                # [123,16] raw query-stream input
_alloc("xk", D)                # [123,16] raw key-stream input
_alloc("pos", D)               # [123,16] positional embedding
_alloc("lnfg", D)              # [123,16] final-LN gain, broadcast
_alloc("lnfb", D)              # [123,16] final-LN bias, broadcast
for b, (l0, nl) in enumerate(BATCH):
    _alloc(f"wkbB{b}", 32 * nl)            # [17, 32nl]
    _alloc(f"wvbB{b}", 32 * nl)            # [17, 32nl] (+ones-row selectors)
    _alloc(f"wobB{b}", 16 * nl)            # [32nl, 16nl] block-diag
for l in range(L):
    _alloc(f"wqbT{l}", D + 1)  # [16,17] at ROWS 32j (j = in-batch index)
for l in range(L):
    _alloc(f"fc1b{l}", FF + 1)  # [17,65] (fc1';b1') + unit col -> ones row
    _alloc(f"fc2b{l}", D)      # [65,16]
for t in range(NTAP):
    _alloc(f"cw{t}", 96)       # [17,96] conv tap t, conv j at cols 32j
_alloc("fc1w97", 30)           # [97,30]: rows 32j+r = fc1_w[10j+r], row96=b
_alloc("fc2w33", OC)           # [33,10]: rows 0-29 fc2_w, row 32 = b
NCOL = _ncol

E16 = np.zeros((17, 1))
E16[16, 0] = 1.0


def _pos_embed() -> np.ndarray:
    half = D // 2
    freqs = np.exp(np.arange(half) * (-np.log(10000.0) / (half - 1)))
    ang = np.arange(SEQ)[:, None] * freqs[None, :]
    return np.concatenate([np.sin(ang), np.cos(ang)], axis=1).astype(np.float32)


def _pack_core(xq: np.ndarray, xk: np.ndarray, enc: dict, conv: dict) -> np.ndarray:
    """Build the [128, NCOL] constants tensor for one core (float64 folding)."""
    cp = np.zeros((128, NCOL), np.float64)

    def put(name: str, arr: np.ndarray) -> None:
        c0, w = _cols[name]
        assert arr.shape[1] == w, (name, arr.shape, w)
        cp[: arr.shape[0], c0:c0 + w] = arr

    put("ident", np.eye(SEQ))
    put("xq", xq.astype(np.float64))
    put("xk", xk.astype(np.float64))
    put("pos", _pos_embed().astype(np.float64))
    put("lnfg", np.broadcast_to(enc["lnf_g"], (SEQ, D)))
    put("lnfb", np.broadcast_to(enc["lnf_b"], (SEQ, D)))

    def fold(l):
        g0, b0 = enc["ln0_g"][l], enc["ln0_b"][l]
        gk, bk_ = enc["ln0k_g"][l], enc["ln0k_b"][l]
        wq, wk, wv, wo = enc["Wq"][l], enc["Wk"][l], enc["Wv"][l], enc["Wo"][l]
        wqb = np.vstack([g0[:, None] * wq * ISC,
                         (b0 @ wq + enc["bq"][l])[None] * ISC])      # [17,16]
        wkb = np.vstack([gk[:, None] * wk,
                         (bk_ @ wk + enc["bk"][l])[None]])           # [17,16]
        wvb = np.vstack([gk[:, None] * wv,
                         (bk_ @ wv + enc["bv"][l])[None]])           # [17,16]
        wob = np.vstack([wo, enc["bo"][l][None]])                    # [17,16]
        return wqb, wkb, wvb, wob

    for b, (l0, nl) in enumerate(BATCH):
        wkbB = np.zeros((17, 32 * nl))
        wvbB = np.zeros((17, 32 * nl))
        wobB = np.zeros((32 * nl, 16 * nl))
        for j in range(nl):
            wqb, wkb, wvb, wob = fold(l0 + j)
            wkbB[:, 32 * j:32 * j + 16] = wkb
            wvbB[:, 32 * j:32 * j + 16] = wvb
            wvbB[16, 32 * j + 16] = 1.0      # selects knT ones row -> vT ones
            wobB[32 * j:32 * j + 17, 16 * j:16 * j + 16] = wob
            # M_l = wqb @ kT_l: lhsT[k,i] = wqb[i,k], placed at rows 32j to
            # match kTall's base partition for the per-layer M matmul
            c0, w = _cols[f"wqbT{l0 + j}"]
            cp[32 * j:32 * j + 16, c0:c0 + w] = wqb.T
        put(f"wkbB{b}", wkbB)
        put(f"wvbB{b}", wvbB)
        put(f"wobB{b}", wobB)

    for l in range(L):
        g1, b1 = enc["ln1_g"][l], enc["ln1_b"][l]
        f1 = np.vstack([g1[:, None] * enc["fc1_w"][l],
                        (b1 @ enc["fc1_w"][l] + enc["fc1_b"][l])[None]])
        put(f"fc1b{l}", np.hstack([f1, E16]))                        # [17,65]
        put(f"fc2b{l}", np.vstack([enc["fc2_w"][l], enc["fc2_b"][l][None]]))

    # conv taps: cw_t[ic, 32j+oc] = w_j[oc, ic, t] (0 when t >= k_j);
    # bias row 16 only on tap 0 (multiplied by the enc ones-row).
    for t in range(NTAP):
        cw = np.zeros((17, 96))
        for j, k in enumerate(KS):
            if t < k:
                cw[:D, 32 * j:32 * j + OC] = conv[f"w{j+1}"][:, :, t].T
            if t == 0:
                cw[D, 32 * j:32 * j + OC] = conv[f"b{j+1}"]
        put(f"cw{t}", cw)
    f97 = np.zeros((97, 30))
    for j in range(3):
        f97[32 * j:32 * j + OC] = conv["fc1_w"][OC * j:OC * j + OC]
    f97[96] = conv["fc1_b"]
    put("fc1w97", f97)
    f33 = np.zeros((33, OC))
    f33[0:30] = conv["fc2_w"]
    f33[32] = conv["fc2_b"]
    put("fc2w33", f33)
    return np.ascontiguousarray(cp, np.float32)


# ------------------------------------------------------------- device build --
def _ln_block(nc, sb, st, x_ap, sum_ap, out_ap, tag, eps_ap):
    """LayerNorm standardize: out = (x - mean(x)) * rsqrt(var(x) + eps).
    `sum_ap` is the precomputed row-sum of x (captured for free via
    accum_out on the op that materialized x). Gains/biases are folded into
    downstream weights by the host. rsqrt = exp(-0.5*ln(v+eps))."""
    negmean = st.tile([SEQ, 1], F32, tag=f"nm{tag}", name=f"nm{tag}")
    nc.vector.tensor_scalar_mul(negmean[:], sum_ap, -1.0 / D)
    ssq = st.tile([SEQ, 1], F32, tag=f"sq{tag}", name=f"sq{tag}")
    scr2 = sb.tile([SEQ, D], F32, tag="scr", bufs=2, name=f"scr{tag}")
    # DVE: out = (x - mean) * x, accum = sum((x-m)x) = sum((x-m)^2)
    nc.vector.scalar_tensor_tensor(scr2[:], x_ap, negmean[:], x_ap,
                                   op0=ALU.add, op1=ALU.mult,
                                   accum_out=ssq[:])
    lnv = st.tile([SEQ, 1], F32, tag=f"lv{tag}", name=f"lv{tag}")
    nc.scalar.activation(lnv[:], ssq[:], AF.Ln, scale=1.0 / D, bias=eps_ap)
    rstd = st.tile([SEQ, 1], F32, tag=f"rs{tag}", name=f"rs{tag}")
    nc.scalar.activation(rstd[:], lnv[:], AF.Exp, scale=-0.5)
    # DVE: out = (x + negmean) * rstd
    nc.vector.tensor_scalar(out_ap, x_ap, negmean[:], rstd[:],
                            op0=ALU.add, op1=ALU.mult)


def _build_nc():
    nc = bacc.Bacc("TRN2", target_bir_lowering=False, debug=False,
                   num_devices=8)
    cp_d = nc.dram_tensor("cpack", [128, NCOL], F32, kind="ExternalInput")
    attn_d = nc.dram_tensor("attn", [SEQ, SEQ], F32, kind="ExternalOutput")
    fch_d = nc.dram_tensor("fch", [FF, SEQ], F32, kind="ExternalOutput")
    fco_d = nc.dram_tensor("fco", [SEQ, D], F32, kind="ExternalOutput")
    enc_d = nc.dram_tensor("enc", [SEQ, D], F32, kind="ExternalOutput")
    feat_d = nc.dram_tensor("feat", [1, OC], F32, kind="ExternalOutput")

    with tile.TileContext(nc, trace_sim=False) as tc, ExitStack() as ctx:
        const = ctx.enter_context(tc.tile_pool(name="const", bufs=1))
        pers = ctx.enter_context(tc.tile_pool(name="pers", bufs=1))
        sb = ctx.enter_context(tc.tile_pool(name="sb", bufs=2))
        st = ctx.enter_context(tc.tile_pool(name="st", bufs=2))
        ps = ctx.enter_context(tc.tile_pool(name="ps", bufs=3, space="PSUM"))
        psb = ctx.enter_context(tc.tile_pool(name="psb", bufs=2, space="PSUM"))
        pso = ctx.enter_context(tc.tile_pool(name="pso", bufs=2, space="PSUM"))

        cp = const.tile([128, NCOL], F32, tag="cp", name="cp")
        nc.sync.dma_start(cp[:], cp_d.ap())

        def C(name, rows):
            c0, w = _cols[name]
            return cp[0:rows, c0:c0 + w]

        ident = C("ident", SEQ)

        # persistent standardized-activation tiles with a ones column 16
        # (transposing them yields the ones-augmented [17,123] layout)
        kstd = pers.tile([SEQ, D + 1], F32, tag="kstd", name="kstd")
        xstd = pers.tile([SEQ, D + 1], F32, tag="xstd", name="xstd")
        l1std = pers.tile([SEQ, D + 1], F32, tag="l1std", name="l1std")
        enc_s = pers.tile([SEQ, D + 1], F32, tag="encs", name="enc_s")
        for t_ in (kstd, xstd, l1std, enc_s):
            nc.vector.memset(t_[:, D:D + 1], 1.0)
        epsv = pers.tile([SEQ, 1], F32, tag="epsv", name="epsv")
        nc.vector.memset(epsv[:], EPS)
        tvec = pers.tile([97, 1], F32, tag="tvec", name="tvec")
        nc.vector.memset(tvec[:], 0.0)
        nc.vector.memset(tvec[96:97, :], 1.0)
        hvv = pers.tile([33, 1], F32, tag="hvv", name="hvv")
        nc.vector.memset(hvv[:], 0.0)
        nc.vector.memset(hvv[32:33, :], 1.0)

        # x0 = 4*xq + POS ; xk0 = 4*xk + POS (row-sums captured for the LNs)
        x = pers.tile([SEQ, D], F32, tag="x0", name="x0")
        xsum = st.tile([SEQ, 1], F32, tag="xsum", bufs=3, name="xsum0")
        nc.vector.scalar_tensor_tensor(x[:], C("xq", SEQ), SCL, C("pos", SEQ),
                                       op0=ALU.mult, op1=ALU.add,
                                       accum_out=xsum[:])
        xk0 = pers.tile([SEQ, D], F32, tag="xk0", name="xk0")
        ksum = st.tile([SEQ, 1], F32, tag="ksum", bufs=1, name="ksum")
        nc.vector.scalar_tensor_tensor(xk0[:], C("xk", SEQ), SCL,
                                       C("pos", SEQ), op0=ALU.mult,
                                       op1=ALU.add, accum_out=ksum[:])

        # key-stream standardization (constant across layers: LN g/b folded)
        _ln_block(nc, sb, st, xk0[:], ksum[:], kstd[:, 0:D], "k", epsv[:])
        ksT_ps = ps.tile([D + 1, SEQ], F32, tag="psK", name="ksT_ps")
        nc.tensor.transpose(ksT_ps[:], kstd[:], ident)
        knT = sb.tile([D + 1, SEQ], F32, tag="knT", bufs=1, name="knT")
        nc.scalar.copy(knT[:], ksT_ps[:])

        # ---- K-side projections for ALL layers, batched (off-path) ----
        # kTall rows 32j..+15 = kT of layer l0+j ; Mall rows 32j..+16 = M_l
        # vTall rows 32j..+16 = vT_l (+ones) ; voall cols 16j..+15 = vo_l
        m_lyr, vo_all = [], []
        for b, (l0, nl) in enumerate(BATCH):
            kT_ps = ps.tile([32 * nl, SEQ], F32, tag="psK", name=f"kT_ps{b}")
            nc.tensor.matmul(kT_ps[:], C(f"wkbB{b}", D + 1), knT[:],
                             start=True, stop=True)
            kT_s = sb.tile([32 * nl, SEQ], F32, tag="kT", name=f"kT_s{b}")
            nc.vector.tensor_copy(kT_s[:], kT_ps[:])
            for j in range(nl):
                c0, w = _cols[f"wqbT{l0 + j}"]
                m_ps = ps.tile([D + 1, SEQ], F32, tag="psK",
                               name=f"m_ps{l0 + j}")
                nc.tensor.matmul(m_ps[:],
                                 cp[32 * j:32 * j + 16, c0:c0 + w],
                                 kT_s[32 * j:32 * j + 16, :], start=True,
                                 stop=True)
                m_s = sb.tile([D + 1, SEQ], F32, tag="m", bufs=5,
                              name=f"m_s{l0 + j}")
                nc.vector.tensor_copy(m_s[:], m_ps[:])
                m_lyr.append(m_s)

            vT_ps = ps.tile([32 * nl, SEQ], F32, tag="psK", name=f"vT_ps{b}")
            nc.tensor.matmul(vT_ps[:], C(f"wvbB{b}", D + 1), knT[:],
                             start=True, stop=True)
            vT_s = sb.tile([32 * nl, SEQ], F32, tag="vT", name=f"vT_s{b}")
            nc.scalar.copy(vT_s[:], vT_ps[:])
            vo_ps = pso.tile([SEQ, 16 * nl], F32, tag="pso", name=f"vo_ps{b}")
            nc.tensor.matmul(vo_ps[:], vT_s[:], C(f"wobB{b}", 32 * nl),
                             start=True, stop=True)
            vo_s = sb.tile([SEQ, 16 * nl], F32, tag="vo", name=f"vo_s{b}")
            nc.scalar.copy(vo_s[:], vo_ps[:])
            vo_all.append(vo_s)

        attn_keep = None
        fco_s = None
        fch_keep = None
        for l in range(L):
            b, j = (0, l) if l < 3 else (1, l - 3)
            # ---- critical path: Q-side LN -> scores -> softmax -> out ----
            _ln_block(nc, sb, st, x[:], xsum[:], xstd[:, 0:D], f"q{l}",
                      epsv[:])
            xsT_ps = ps.tile([D + 1, SEQ], F32, tag="psK", name=f"xsT_ps{l}")
            nc.tensor.transpose(xsT_ps[:], xstd[:], ident)
            xsT_s = sb.tile([D + 1, SEQ], F32, tag="xsT", name=f"xsT_s{l}")
            nc.scalar.copy(xsT_s[:], xsT_ps[:])
            sc_ps = psb.tile([SEQ, SEQ], F32, tag="big", name=f"sc_ps{l}")
            nc.tensor.matmul(sc_ps[:], xsT_s[:], m_lyr[l][:], start=True,
                             stop=True)
            # softmax: no max-subtraction (scores are O(1); shift-invariant),
            # normalization folded into the residual add via 1/rowsum
            rowsum = st.tile([SEQ, 1], F32, tag="rsum", name=f"rowsum{l}")
            e_s = sb.tile([SEQ, SEQ], F32, tag="e", name=f"e_s{l}")
            nc.scalar.activation(e_s[:], sc_ps[:], AF.Exp,
                                 accum_out=rowsum[:])
            rinv = st.tile([SEQ, 1], F32, tag="rinv", name=f"rinv{l}")
            nc.vector.reciprocal(rinv[:], rowsum[:])
            eT_ps = psb.tile([SEQ, SEQ], F32, tag="big", name=f"eT_ps{l}")
            nc.tensor.transpose(eT_ps[:], e_s[:], ident)
            eT_s = sb.tile([SEQ, SEQ], F32, tag="eT", name=f"eT_s{l}")
            nc.scalar.copy(eT_s[:], eT_ps[:])
            o_ps = pso.tile([SEQ, D], F32, tag="pso", name=f"o_ps{l}")
            nc.tensor.matmul(o_ps[:], eT_s[:],
                             vo_all[b][:, 16 * j:16 * j + 16], start=True,
                             stop=True)
            x2 = sb.tile([SEQ, D], F32, tag="x", bufs=3, name=f"x2_{l}")
            x2sum = st.tile([SEQ, 1], F32, tag="xsum", bufs=3,
                            name=f"x2sum{l}")
            # x2 = (E @ vo) * (1/rowsum) + x  == attn @ vo + x
            nc.vector.scalar_tensor_tensor(x2[:], o_ps[:], rinv[:], x[:],
                                           op0=ALU.mult, op1=ALU.add,
                                           accum_out=x2sum[:])
            if l == L - 1:
                attn_keep = sb.tile([SEQ, SEQ], F32, tag="attn", bufs=1,
                                    name="attn_keep")
                nc.vector.tensor_scalar_mul(attn_keep[:], e_s[:], rinv[:])

            # ---- FFN ----
            _ln_block(nc, sb, st, x2[:], x2sum[:], l1std[:, 0:D], f"f{l}",
                      epsv[:])
            l1T_ps = ps.tile([D + 1, SEQ], F32, tag="psK", name=f"l1T_ps{l}")
            nc.tensor.transpose(l1T_ps[:], l1std[:], ident)
            l1T_s = sb.tile([D + 1, SEQ], F32, tag="l1T", name=f"l1T_s{l}")
            nc.scalar.copy(l1T_s[:], l1T_ps[:])
            hT_ps = pso.tile([FF + 1, SEQ], F32, tag="pso", name=f"hT_ps{l}")
            nc.tensor.matmul(hT_ps[:], C(f"fc1b{l}", D + 1), l1T_s[:],
                             start=True, stop=True)
            hT_s = sb.tile([FF + 1, SEQ], F32, tag="hT", name=f"hT_s{l}")
            nc.scalar.activation(hT_s[:], hT_ps[:], AF.Relu)
            fco_ps = pso.tile([SEQ, D], F32, tag="pso", name=f"fco_ps{l}")
            nc.tensor.matmul(fco_ps[:], hT_s[:], C(f"fc2b{l}", FF + 1),
                             start=True, stop=True)
            if l == L - 1:
                fco_s = sb.tile([SEQ, D], F32, tag="fcos", bufs=1,
                                name="fco_s")
                nc.vector.tensor_copy(fco_s[:], fco_ps[:])
                fch_keep = hT_s
            x3 = sb.tile([SEQ, D], F32, tag="x", bufs=3, name=f"x3_{l}")
            xsum = st.tile([SEQ, 1], F32, tag="xsum", bufs=3,
                           name=f"x3sum{l}")
            nc.vector.scalar_tensor_tensor(x3[:], fco_ps[:], 1.0, x2[:],
                                           op0=ALU.mult, op1=ALU.add,
                                           accum_out=xsum[:])
            x = x3

        # ---- final LN with real gain/bias, encoder output ----
        xfstd = sb.tile([SEQ, D], F32, tag="xfstd", bufs=1, name="xfstd")
        _ln_block(nc, sb, st, x[:], xsum[:], xfstd[:], "fin", epsv[:])
        enc_g = sb.tile([SEQ, D], F32, tag="encg", bufs=1, name="enc_g")
        nc.vector.tensor_mul(enc_g[:], xfstd[:], C("lnfg", SEQ))
        nc.vector.tensor_add(enc_s[:, 0:D], enc_g[:], C("lnfb", SEQ))

        # encoder outputs -> DRAM
        nc.sync.dma_start(attn_d.ap(), attn_keep[:])
        nc.sync.dma_start(fch_d.ap(), fch_keep[0:FF, :])
        nc.sync.dma_start(fco_d.ap(), fco_s[:])
        nc.sync.dma_start(enc_d.ap(), enc_s[:, 0:D])

        # ---- conv branch on this core's encoder output ----
        encT_ps = ps.tile([D + 1, SEQ], F32, tag="psK", name="encT_ps")
        nc.tensor.transpose(encT_ps[:], enc_s[:], ident)
        # zero-padded to width SEQ+9 so every conv tap covers the full
        # [96,123] PSUM region (well-formed accumulation group)
        encT_s = sb.tile([D + 1, SEQ + NTAP - 1], F32, tag="encT", bufs=1,
                         name="encT_s")
        nc.vector.memset(encT_s[:, SEQ:], 0.0)
        nc.scalar.copy(encT_s[:, 0:SEQ], encT_ps[:])
        # conv j output rows at 32j..32j+9 (quadrant-aligned)
        cv_ps = psb.tile([96, SEQ], F32, tag="big", name="cv_ps")
        for t in range(NTAP):
            nc.tensor.matmul(cv_ps[:], C(f"cw{t}", D + 1),
                             encT_s[:, t:t + SEQ], start=(t == 0),
                             stop=(t == NTAP - 1))
        # max over valid windows straight from PSUM; relu after max
        # (relu(max(x)) == max(relu(x)) since relu is monotone)
        for j, k in enumerate(KS):
            nc.vector.reduce_max(tvec[32 * j:32 * j + OC, :],
                                 cv_ps[32 * j:32 * j + OC, 0:SEQ - k + 1],
                                 axis=AX.X)
        tvr = sb.tile([97, 1], F32, tag="tvr", bufs=1, name="tvr")
        nc.scalar.activation(tvr[:], tvec[:], AF.Relu)
        # branch fc1 (bias via ones row 96): ftT = fc1w97.T @ tvr [30,1]
        ftT_ps = pso.tile([30, 1], F32, tag="pso", name="ftT_ps")
        nc.tensor.matmul(ftT_ps[:], C("fc1w97", 97), tvr[:], start=True,
                         stop=True)
        nc.scalar.activation(hvv[0:30, :], ftT_ps[:], AF.Relu)
        feat_ps = pso.tile([1, OC], F32, tag="pso", name="feat_ps")
        nc.tensor.matmul(feat_ps[:], hvv[:], C("fc2w33", 33), start=True,
                         stop=True)
        feat_s = sb.tile([1, OC], F32, tag="feat", bufs=1, name="feat_s")
        nc.vector.tensor_copy(feat_s[:], feat_ps[:])
        nc.sync.dma_start(feat_d.ap(), feat_s[:])

    nc.compile()
    return nc


_NC_CACHE: list = []


def _get_nc():
    if not _NC_CACHE:
        _NC_CACHE.append(_build_nc())
    return _NC_CACHE[0]


# ------------------------------------------------------------------- driver --
def _np_tree(obj):
    if isinstance(obj, dict):
        return {k: _np_tree(v) for k, v in obj.items()}
    return np.asarray(obj)


def _prep(x, params):
    """Host-side input prep: build the 8 per-core constant packs."""
    x = np.asarray(x)
    p = _np_tree(params)

    wavA = x[0, 0, 0]
    wavB = x[0, 0, -1]
    eeg = x[0, 0, 1:-1, TD:WIN - 1]                  # (16, SEQ)
    idx = (np.arange(TD)[:, None] + 1) + np.arange(SEQ)[None, :]
    wA = np.ascontiguousarray(wavA[idx].T)           # (SEQ, 16)
    wB = np.ascontiguousarray(wavB[idx].T)
    eeg_t = np.ascontiguousarray(eeg.T)              # (SEQ, 16)

    def conv_params(i):
        return dict(w1=p["conv1_w"][i], b1=p["conv1_b"][i],
                    w2=p["conv2_w"][i], b2=p["conv2_b"][i],
                    w3=p["conv3_w"][i], b3=p["conv3_b"][i],
                    fc1_w=p["fc1_w"][i], fc1_b=p["fc1_b"][i],
                    fc2_w=p["fc2_w"][i], fc2_b=p["fc2_b"][i])

    # core -> (encoder params, xq, xk, conv branch index)
    plan = [
        (p["a2e"], wA, eeg_t, 0),     # wavA_t  / branch 0
        (p["e2a"], eeg_t, wA, 1),     # eegA    / branch 1
        (p["e2a2"], eeg_t, wB, 2),    # eegB    / branch 2
        (p["a2e2"], wB, eeg_t, 3),    # wavB_t  / branch 3
    ]
    packs = [_pack_core(xq, xk, encp, conv_params(bi))
             for encp, xq, xk, bi in plan]
    in_maps = [{"cpack": packs[c % 4]} for c in range(8)]
    return in_maps, wA


def _host_head(feats, p):
    """Tiny 2-layer sigmoid head (1.7 KFLOP) on the gathered features."""
    z = np.concatenate([f.astype(np.float64) for f in feats], axis=1)
    s1 = 1.0 / (1.0 + np.exp(-(z @ p["out_w1"].astype(np.float64)
                               + p["out_b1"].astype(np.float64))))
    o = 1.0 / (1.0 + np.exp(-(s1 @ p["out_w2"].astype(np.float64)
                              + p["out_b2"].astype(np.float64))))
    return o.astype(np.float32)


def kernel(x, params):
    in_maps, wA = _prep(x, params)
    nc = _get_nc()
    res = run_bass_kernel_spmd(nc, in_maps, list(range(8))).results

    p = _np_tree(params)
    out = _host_head([res[c]["feat"] for c in range(4)], p)
    aeA = res[0]["attn"]
    eaA = res[1]["attn"]
    eaB = res[2]["attn"]
    aeB = res[3]["attn"]
    wavA_before = wA[None].astype(np.float32)
    wavA_after = res[0]["enc"][None]
    ae_fcA = np.ascontiguousarray(res[0]["fch"].T)
    ae_fc_lastA = res[0]["fco"]
    return (out, aeA, (wavA_before, wavA_after), ae_fcA, eaA, aeB, eaB,
            ae_fc_lastA)


# revision 27
# speedup vs baseline: 1.9775x; 1.0091x over previous
"""Trainium2 Bass kernel for the MulT-style cross-modal CNN/transformer.

Strategy (SPMD over 8 NeuronCores):
  - The model contains 4 independent 5-layer cross-modal encoders followed by
    4 independent conv branches and a tiny shared head. Core c (c in 0..3)
    runs encoder c and conv branch c on its own encoder output; cores 4..7
    mirror cores 0..3 (same program, same data -> harmless redundancy).
  - Branch features (1,10) are per-core outputs; the 1.7-KFLOP sigmoid head
    runs on the host (collectives cost ~80us of latency for 160 bytes).
  - Host-side prep folds all LayerNorm gains/biases into the adjacent weight
    matrices, folds biases into ones-row augmented matmuls, pre-applies the
    attention output projection to V (vo = v @ Wo + bo), and packs every
    constant this core needs into ONE [128, NCOL] f32 tensor -> one DMA.
  - PE instruction count dominates (fixed ~400ns/matmul at these sizes), so
    the K-side projections of all 5 layers are batched into 32-partition-
    aligned block matmuls, the 3 convs into one 32-aligned block layout, and
    softmax normalization is folded into the residual add (1/rowsum scaling)
    so it leaves the critical path.
  - On-device transcendentals use only the `natural_log_exp_and_others`
    activation table (copy/relu/ln/exp): rsqrt(v) = exp(-0.5*ln(v+eps)) ->
    exactly one ACT table load per run.
  - All SBUF/PSUM access patterns start at partition offsets that are
    multiples of 32 (HW quadrant constraint).
"""

import numpy as np
from contextlib import ExitStack

import concourse.bass as bass
import concourse.tile as tile
from concourse import bacc, mybir
from concourse.bass_utils import run_bass_kernel_spmd

# The act-table chooser statically maps each activation function to the
# first table set containing it (Exp -> exp_and_others, Ln -> natural_log),
# which thrashes ACT_TABLE_LOAD (~1.3us each) on every Ln/Exp alternation.
# Every function this kernel uses (copy/relu/ln/exp) lives in the
# natural_log set, so mask all other sets (preserving dict order, hence
# act_func_set_id indices) to get exactly one table load per run.
_orig_gat = bacc.get_activation_tables


def _gat_single_set(arch):
    out = {}
    for name, fns in _orig_gat(arch).items():
        out[name] = fns if name.startswith("natural_log") else set()
    return out


bacc.get_activation_tables = _gat_single_set

F32 = mybir.dt.float32
AF = mybir.ActivationFunctionType
ALU = mybir.AluOpType
AX = mybir.AxisListType

SEQ, D, L, FF, OC, NB = 123, 16, 5, 64, 10, 4
WIN, TD = 140, 16
KS = (8, 9, 10)            # conv kernel sizes
NTAP = max(KS)             # 10 accumulation taps
EPS = 1e-5
SCL = float(D) ** 0.5      # 4.0
ISC = float(D) ** -0.5     # 0.25
# layer batches for the K-side block matmuls: (start layer, n layers)
BATCH = ((0, 3), (3, 2))

# ------------------------------------------------------------------ layout --
_cols: dict[str, tuple[int, int]] = {}
_ncol = 0


def _alloc(name: str, cols: int) -> None:
    global _ncol
    _cols[name] = (_ncol, cols)
    _ncol += cols


_alloc("ident", SEQ)           # [123,123] identity
_alloc("xq", D)                # [123,16] raw query-stream input
_alloc("xk", D)                # [123,16] raw key-stream input
_alloc("pos", D)               # [123,16] positional embedding
_alloc("lnfg", D)              # [123,16] final-LN gain, broadcast
_alloc("lnfb", D)              # [123,16] final-LN bias, broadcast
for b, (l0, nl) in enumerate(BATCH):
    _alloc(f"wkbB{b}", 32 * nl)            # [17, 32nl]
    _alloc(f"wvbB{b}", 32 * nl)            # [17, 32nl] (+ones-row selectors)
    _alloc(f"wobB{b}", 16 * nl)            # [32nl, 16nl] block-diag
for l in range(L):
    _alloc(f"wqbT{l}", D + 1)  # [16,17] at ROWS 32j (j = in-batch index)
for l in range(L):
    _alloc(f"fc1b{l}", FF + 1)  # [17,65] (fc1';b1') + unit col -> ones row
    _alloc(f"fc2b{l}", D)      # [65,16]
for t in range(NTAP):
    _alloc(f"cw{t}", 96)       # [17,96] conv tap t, conv j at cols 32j
_alloc("fc1w97", 30)           # [97,30]: rows 32j+r = fc1_w[10j+r], row96=b
_alloc("fc2w33", OC)           # [33,10]: rows 0-29 fc2_w, row 32 = b
NCOL = _ncol

E16 = np.zeros((17, 1))
E16[16, 0] = 1.0


def _pos_embed() -> np.ndarray:
    half = D // 2
    freqs = np.exp(np.arange(half) * (-np.log(10000.0) / (half - 1)))
    ang = np.arange(SEQ)[:, None] * freqs[None, :]
    return np.concatenate([np.sin(ang), np.cos(ang)], axis=1).astype(np.float32)


def _pack_core(xq: np.ndarray, xk: np.ndarray, enc: dict, conv: dict) -> np.ndarray:
    """Build the [128, NCOL] constants tensor for one core (float64 folding)."""
    cp = np.zeros((128, NCOL), np.float64)

    def put(name: str, arr: np.ndarray) -> None:
        c0, w = _cols[name]
        assert arr.shape[1] == w, (name, arr.shape, w)
        cp[: arr.shape[0], c0:c0 + w] = arr

    put("ident", np.eye(SEQ))
    put("xq", xq.astype(np.float64))
    put("xk", xk.astype(np.float64))
    put("pos", _pos_embed().astype(np.float64))
    put("lnfg", np.broadcast_to(enc["lnf_g"], (SEQ, D)))
    put("lnfb", np.broadcast_to(enc["lnf_b"], (SEQ, D)))

    def fold(l):
        g0, b0 = enc["ln0_g"][l], enc["ln0_b"][l]
        gk, bk_ = enc["ln0k_g"][l], enc["ln0k_b"][l]
        wq, wk, wv, wo = enc["Wq"][l], enc["Wk"][l], enc["Wv"][l], enc["Wo"][l]
        wqb = np.vstack([g0[:, None] * wq * ISC,
                         (b0 @ wq + enc["bq"][l])[None] * ISC])      # [17,16]
        wkb = np.vstack([gk[:, None] * wk,
                         (bk_ @ wk + enc["bk"][l])[None]])           # [17,16]
        wvb = np.vstack([gk[:, None] * wv,
                         (bk_ @ wv + enc["bv"][l])[None]])           # [17,16]
        wob = np.vstack([wo, enc["bo"][l][None]])                    # [17,16]
        return wqb, wkb, wvb, wob

    for b, (l0, nl) in enumerate(BATCH):
        wkbB = np.zeros((17, 32 * nl))
        wvbB = np.zeros((17, 32 * nl))
        wobB = np.zeros((32 * nl, 16 * nl))
        for j in range(nl):
            wqb, wkb, wvb, wob = fold(l0 + j)
            wkbB[:, 32 * j:32 * j + 16] = wkb
            wvbB[:, 32 * j:32 * j + 16] = wvb
            wvbB[16, 32 * j + 16] = 1.0      # selects knT ones row -> vT ones
            wobB[32 * j:32 * j + 17, 16 * j:16 * j + 16] = wob
            # M_l = wqb @ kT_l: lhsT[k,i] = wqb[i,k], placed at rows 32j to
            # match kTall's base partition for the per-layer M matmul
            c0, w = _cols[f"wqbT{l0 + j}"]
            cp[32 * j:32 * j + 16, c0:c0 + w] = wqb.T
        put(f"wkbB{b}", wkbB)
        put(f"wvbB{b}", wvbB)
        put(f"wobB{b}", wobB)

    for l in range(L):
        g1, b1 = enc["ln1_g"][l], enc["ln1_b"][l]
        f1 = np.vstack([g1[:, None] * enc["fc1_w"][l],
                        (b1 @ enc["fc1_w"][l] + enc["fc1_b"][l])[None]])
        put(f"fc1b{l}", np.hstack([f1, E16]))                        # [17,65]
        put(f"fc2b{l}", np.vstack([enc["fc2_w"][l], enc["fc2_b"][l][None]]))

    # conv taps: cw_t[ic, 32j+oc] = w_j[oc, ic, t] (0 when t >= k_j);
    # bias row 16 only on tap 0 (multiplied by the enc ones-row).
    for t in range(NTAP):
        cw = np.zeros((17, 96))
        for j, k in enumerate(KS):
            if t < k:
                cw[:D, 32 * j:32 * j + OC] = conv[f"w{j+1}"][:, :, t].T
            if t == 0:
                cw[D, 32 * j:32 * j + OC] = conv[f"b{j+1}"]
        put(f"cw{t}", cw)
    f97 = np.zeros((97, 30))
    for j in range(3):
        f97[32 * j:32 * j + OC] = conv["fc1_w"][OC * j:OC * j + OC]
    f97[96] = conv["fc1_b"]
    put("fc1w97", f97)
    f33 = np.zeros((33, OC))
    f33[0:30] = conv["fc2_w"]
    f33[32] = conv["fc2_b"]
    put("fc2w33", f33)
    return np.ascontiguousarray(cp, np.float32)


# ------------------------------------------------------------- device build --
def _ln_block(nc, sb, st, x_ap, sum_ap, out_ap, tag, eps_ap):
    """LayerNorm standardize: out = (x - mean(x)) * rsqrt(var(x) + eps).
    `sum_ap` is the precomputed row-sum of x (captured for free via
    accum_out on the op that materialized x). Gains/biases are folded into
    downstream weights by the host. rsqrt = exp(-0.5*ln(v+eps))."""
    negmean = st.tile([SEQ, 1], F32, tag=f"nm{tag}", name=f"nm{tag}")
    nc.vector.tensor_scalar_mul(negmean[:], sum_ap, -1.0 / D)
    ssq = st.tile([SEQ, 1], F32, tag=f"sq{tag}", name=f"sq{tag}")
    scr2 = sb.tile([SEQ, D], F32, tag="scr", bufs=2, name=f"scr{tag}")
    # DVE: out = (x - mean) * x, accum = sum((x-m)x) = sum((x-m)^2)
    nc.vector.scalar_tensor_tensor(scr2[:], x_ap, negmean[:], x_ap,
                                   op0=ALU.add, op1=ALU.mult,
                                   accum_out=ssq[:])
    lnv = st.tile([SEQ, 1], F32, tag=f"lv{tag}", name=f"lv{tag}")
    nc.scalar.activation(lnv[:], ssq[:], AF.Ln, scale=1.0 / D, bias=eps_ap)
    rstd = st.tile([SEQ, 1], F32, tag=f"rs{tag}", name=f"rs{tag}")
    nc.scalar.activation(rstd[:], lnv[:], AF.Exp, scale=-0.5)
    # DVE: out = (x + negmean) * rstd
    nc.vector.tensor_scalar(out_ap, x_ap, negmean[:], rstd[:],
                            op0=ALU.add, op1=ALU.mult)


def _build_nc():
    nc = bacc.Bacc("TRN2", target_bir_lowering=False, debug=False,
                   num_devices=8)
    cp_d = nc.dram_tensor("cpack", [128, NCOL], F32, kind="ExternalInput")
    attn_d = nc.dram_tensor("attn", [SEQ, SEQ], F32, kind="ExternalOutput")
    fch_d = nc.dram_tensor("fch", [FF, SEQ], F32, kind="ExternalOutput")
    fco_d = nc.dram_tensor("fco", [SEQ, D], F32, kind="ExternalOutput")
    enc_d = nc.dram_tensor("enc", [SEQ, D], F32, kind="ExternalOutput")
    feat_d = nc.dram_tensor("feat", [1, OC], F32, kind="ExternalOutput")

    with tile.TileContext(nc, trace_sim=False) as tc, ExitStack() as ctx:
        const = ctx.enter_context(tc.tile_pool(name="const", bufs=1))
        pers = ctx.enter_context(tc.tile_pool(name="pers", bufs=1))
        sb = ctx.enter_context(tc.tile_pool(name="sb", bufs=2))
        st = ctx.enter_context(tc.tile_pool(name="st", bufs=2))
        ps = ctx.enter_context(tc.tile_pool(name="ps", bufs=3, space="PSUM"))
        psb = ctx.enter_context(tc.tile_pool(name="psb", bufs=2, space="PSUM"))
        pso = ctx.enter_context(tc.tile_pool(name="pso", bufs=2, space="PSUM"))

        cp = const.tile([128, NCOL], F32, tag="cp", name="cp")
        nc.sync.dma_start(cp[:], cp_d.ap())

        def C(name, rows):
            c0, w = _cols[name]
            return cp[0:rows, c0:c0 + w]

        ident = C("ident", SEQ)

        def MM(out, lhsT, rhs, **kw):
            nc.tensor.matmul(out, lhsT, rhs, **kw)

        def TR(out, in_):
            nc.tensor.transpose(out, in_, ident)

        # persistent standardized-activation tiles with a ones column 16
        # (transposing them yields the ones-augmented [17,123] layout)
        kstd = pers.tile([SEQ, D + 1], F32, tag="kstd", name="kstd")
        xstd = pers.tile([SEQ, D + 1], F32, tag="xstd", name="xstd")
        l1std = pers.tile([SEQ, D + 1], F32, tag="l1std", name="l1std")
        enc_s = pers.tile([SEQ, D + 1], F32, tag="encs", name="enc_s")
        for t_ in (kstd, xstd, l1std, enc_s):
            nc.vector.memset(t_[:, D:D + 1], 1.0)
        epsv = pers.tile([SEQ, 1], F32, tag="epsv", name="epsv")
        nc.vector.memset(epsv[:], EPS)
        tvec = pers.tile([97, 1], F32, tag="tvec", name="tvec")
        nc.vector.memset(tvec[:], 0.0)
        nc.vector.memset(tvec[96:97, :], 1.0)
        hvv = pers.tile([33, 1], F32, tag="hvv", name="hvv")
        nc.vector.memset(hvv[:], 0.0)
        nc.vector.memset(hvv[32:33, :], 1.0)

        # x0 = 4*xq + POS ; xk0 = 4*xk + POS (row-sums captured for the LNs)
        x = pers.tile([SEQ, D], F32, tag="x0", name="x0")
        xsum = st.tile([SEQ, 1], F32, tag="xsum", bufs=3, name="xsum0")
        nc.vector.scalar_tensor_tensor(x[:], C("xq", SEQ), SCL, C("pos", SEQ),
                                       op0=ALU.mult, op1=ALU.add,
                                       accum_out=xsum[:])
        xk0 = pers.tile([SEQ, D], F32, tag="xk0", name="xk0")
        ksum = st.tile([SEQ, 1], F32, tag="ksum", bufs=1, name="ksum")
        nc.vector.scalar_tensor_tensor(xk0[:], C("xk", SEQ), SCL,
                                       C("pos", SEQ), op0=ALU.mult,
                                       op1=ALU.add, accum_out=ksum[:])

        # key-stream standardization (constant across layers: LN g/b folded)
        _ln_block(nc, sb, st, xk0[:], ksum[:], kstd[:, 0:D], "k", epsv[:])
        ksT_ps = ps.tile([D + 1, SEQ], F32, tag="psK", name="ksT_ps")
        TR(ksT_ps[:], kstd[:])
        knT = sb.tile([D + 1, SEQ], F32, tag="knT", bufs=1, name="knT")
        nc.scalar.copy(knT[:], ksT_ps[:])

        # ---- K-side projections for ALL layers, batched (off-path) ----
        # kTall rows 32j..+15 = kT of layer l0+j ; Mall rows 32j..+16 = M_l
        # vTall rows 32j..+16 = vT_l (+ones) ; voall cols 16j..+15 = vo_l
        m_lyr, vo_all = [], []
        for b, (l0, nl) in enumerate(BATCH):
            kT_ps = ps.tile([32 * nl, SEQ], F32, tag="psK", name=f"kT_ps{b}")
            MM(kT_ps[:], C(f"wkbB{b}", D + 1), knT[:],
                             start=True, stop=True)
            kT_s = sb.tile([32 * nl, SEQ], F32, tag="kT", name=f"kT_s{b}")
            nc.vector.tensor_copy(kT_s[:], kT_ps[:])
            for j in range(nl):
                c0, w = _cols[f"wqbT{l0 + j}"]
                m_ps = ps.tile([D + 1, SEQ], F32, tag="psK",
                               name=f"m_ps{l0 + j}")
                MM(m_ps[:],
                                 cp[32 * j:32 * j + 16, c0:c0 + w],
                                 kT_s[32 * j:32 * j + 16, :], start=True,
                                 stop=True)
                m_s = sb.tile([D + 1, SEQ], F32, tag="m", bufs=5,
                              name=f"m_s{l0 + j}")
                nc.vector.tensor_copy(m_s[:], m_ps[:])
                m_lyr.append(m_s)

            vT_ps = ps.tile([32 * nl, SEQ], F32, tag="psK", name=f"vT_ps{b}")
            MM(vT_ps[:], C(f"wvbB{b}", D + 1), knT[:],
                             start=True, stop=True)
            vT_s = sb.tile([32 * nl, SEQ], F32, tag="vT", name=f"vT_s{b}")
            nc.scalar.copy(vT_s[:], vT_ps[:])
            vo_ps = pso.tile([SEQ, 16 * nl], F32, tag="pso", name=f"vo_ps{b}")
            MM(vo_ps[:], vT_s[:], C(f"wobB{b}", 32 * nl),
                             start=True, stop=True)
            vo_s = sb.tile([SEQ, 16 * nl], F32, tag="vo", name=f"vo_s{b}")
            nc.scalar.copy(vo_s[:], vo_ps[:])
            vo_all.append(vo_s)

        attn_keep = None
        fco_s = None
        fch_keep = None
        for l in range(L):
            b, j = (0, l) if l < 3 else (1, l - 3)
            # ---- critical path: Q-side LN -> scores -> softmax -> out ----
            _ln_block(nc, sb, st, x[:], xsum[:], xstd[:, 0:D], f"q{l}",
                      epsv[:])
            xsT_ps = ps.tile([D + 1, SEQ], F32, tag="psK", name=f"xsT_ps{l}")
            TR(xsT_ps[:], xstd[:])
            xsT_s = sb.tile([D + 1, SEQ], F32, tag="xsT", name=f"xsT_s{l}")
            nc.scalar.copy(xsT_s[:], xsT_ps[:])
            sc_ps = psb.tile([SEQ, SEQ], F32, tag="big", name=f"sc_ps{l}")
            MM(sc_ps[:], xsT_s[:], m_lyr[l][:], start=True,
                             stop=True)
            # softmax: no max-subtraction (scores are O(1); shift-invariant),
            # normalization folded into the residual add via 1/rowsum
            rowsum = st.tile([SEQ, 1], F32, tag="rsum", name=f"rowsum{l}")
            e_s = sb.tile([SEQ, SEQ], F32, tag="e", name=f"e_s{l}")
            nc.scalar.activation(e_s[:], sc_ps[:], AF.Exp)
            nc.vector.reduce_sum(rowsum[:], e_s[:], axis=AX.X)
            rinv = st.tile([SEQ, 1], F32, tag="rinv", name=f"rinv{l}")
            nc.vector.reciprocal(rinv[:], rowsum[:])
            eT_ps = psb.tile([SEQ, SEQ], F32, tag="big", name=f"eT_ps{l}")
            TR(eT_ps[:], e_s[:])
            eT_s = sb.tile([SEQ, SEQ], F32, tag="eT", name=f"eT_s{l}")
            nc.scalar.copy(eT_s[:], eT_ps[:])
            o_ps = pso.tile([SEQ, D], F32, tag="pso", name=f"o_ps{l}")
            MM(o_ps[:], eT_s[:],
                             vo_all[b][:, 16 * j:16 * j + 16], start=True,
                             stop=True)
            x2 = sb.tile([SEQ, D], F32, tag="x", bufs=3, name=f"x2_{l}")
            x2sum = st.tile([SEQ, 1], F32, tag="xsum", bufs=3,
                            name=f"x2sum{l}")
            # x2 = (E @ vo) * (1/rowsum) + x  == attn @ vo + x
            nc.vector.scalar_tensor_tensor(x2[:], o_ps[:], rinv[:], x[:],
                                           op0=ALU.mult, op1=ALU.add,
                                           accum_out=x2sum[:])
            if l == L - 1:
                attn_keep = sb.tile([SEQ, SEQ], F32, tag="attn", bufs=1,
                                    name="attn_keep")
                nc.vector.tensor_scalar_mul(attn_keep[:], e_s[:], rinv[:])

            # ---- FFN ----
            _ln_block(nc, sb, st, x2[:], x2sum[:], l1std[:, 0:D], f"f{l}",
                      epsv[:])
            l1T_ps = ps.tile([D + 1, SEQ], F32, tag="psK", name=f"l1T_ps{l}")
            TR(l1T_ps[:], l1std[:])
            l1T_s = sb.tile([D + 1, SEQ], F32, tag="l1T", name=f"l1T_s{l}")
            nc.scalar.copy(l1T_s[:], l1T_ps[:])
            hT_ps = pso.tile([FF + 1, SEQ], F32, tag="pso", name=f"hT_ps{l}")
            MM(hT_ps[:], C(f"fc1b{l}", D + 1), l1T_s[:],
                             start=True, stop=True)
            hT_s = sb.tile([FF + 1, SEQ], F32, tag="hT", name=f"hT_s{l}")
            nc.scalar.activation(hT_s[:], hT_ps[:], AF.Relu)
            fco_ps = pso.tile([SEQ, D], F32, tag="pso", name=f"fco_ps{l}")
            MM(fco_ps[:], hT_s[:], C(f"fc2b{l}", FF + 1),
                             start=True, stop=True)
            if l == L - 1:
                fco_s = sb.tile([SEQ, D], F32, tag="fcos", bufs=1,
                                name="fco_s")
                nc.vector.tensor_copy(fco_s[:], fco_ps[:])
                fch_keep = hT_s
            x3 = sb.tile([SEQ, D], F32, tag="x", bufs=3, name=f"x3_{l}")
            xsum = st.tile([SEQ, 1], F32, tag="xsum", bufs=3,
                           name=f"x3sum{l}")
            nc.vector.scalar_tensor_tensor(x3[:], fco_ps[:], 1.0, x2[:],
                                           op0=ALU.mult, op1=ALU.add,
                                           accum_out=xsum[:])
            x = x3

        # ---- final LN with real gain/bias, encoder output ----
        xfstd = sb.tile([SEQ, D], F32, tag="xfstd", bufs=1, name="xfstd")
        _ln_block(nc, sb, st, x[:], xsum[:], xfstd[:], "fin", epsv[:])
        enc_g = sb.tile([SEQ, D], F32, tag="encg", bufs=1, name="enc_g")
        nc.vector.tensor_mul(enc_g[:], xfstd[:], C("lnfg", SEQ))
        nc.vector.tensor_add(enc_s[:, 0:D], enc_g[:], C("lnfb", SEQ))

        # encoder outputs -> DRAM
        nc.sync.dma_start(attn_d.ap(), attn_keep[:])
        nc.sync.dma_start(fch_d.ap(), fch_keep[0:FF, :])
        nc.sync.dma_start(fco_d.ap(), fco_s[:])
        nc.sync.dma_start(enc_d.ap(), enc_s[:, 0:D])

        # ---- conv branch on this core's encoder output ----
        encT_ps = ps.tile([D + 1, SEQ], F32, tag="psK", name="encT_ps")
        TR(encT_ps[:], enc_s[:])
        # zero-padded to width SEQ+9 so every conv tap covers the full
        # [96,123] PSUM region (well-formed accumulation group)
        encT_s = sb.tile([D + 1, SEQ + NTAP - 1], F32, tag="encT", bufs=1,
                         name="encT_s")
        nc.vector.memset(encT_s[:, SEQ:], 0.0)
        nc.scalar.copy(encT_s[:, 0:SEQ], encT_ps[:])
        # conv j output rows at 32j..32j+9 (quadrant-aligned)
        cv_ps = psb.tile([96, SEQ], F32, tag="big", name="cv_ps")
        for t in range(NTAP):
            MM(cv_ps[:], C(f"cw{t}", D + 1),
                             encT_s[:, t:t + SEQ], start=(t == 0),
                             stop=(t == NTAP - 1))
        # max over valid windows straight from PSUM; relu after max
        # (relu(max(x)) == max(relu(x)) since relu is monotone)
        for j, k in enumerate(KS):
            nc.vector.reduce_max(tvec[32 * j:32 * j + OC, :],
                                 cv_ps[32 * j:32 * j + OC, 0:SEQ - k + 1],
                                 axis=AX.X)
        tvr = sb.tile([97, 1], F32, tag="tvr", bufs=1, name="tvr")
        nc.scalar.activation(tvr[:], tvec[:], AF.Relu)
        # branch fc1 (bias via ones row 96): ftT = fc1w97.T @ tvr [30,1]
        ftT_ps = pso.tile([30, 1], F32, tag="pso", name="ftT_ps")
        MM(ftT_ps[:], C("fc1w97", 97), tvr[:], start=True,
                         stop=True)
        nc.scalar.activation(hvv[0:30, :], ftT_ps[:], AF.Relu)
        feat_ps = pso.tile([1, OC], F32, tag="pso", name="feat_ps")
        MM(feat_ps[:], hvv[:], C("fc2w33", 33), start=True,
                         stop=True)
        feat_s = sb.tile([1, OC], F32, tag="feat", bufs=1, name="feat_s")
        nc.vector.tensor_copy(feat_s[:], feat_ps[:])
        nc.sync.dma_start(feat_d.ap(), feat_s[:])

    nc.compile()
    return nc


_NC_CACHE: list = []


def _get_nc():
    if not _NC_CACHE:
        _NC_CACHE.append(_build_nc())
    return _NC_CACHE[0]


# ------------------------------------------------------------------- driver --
def _np_tree(obj):
    if isinstance(obj, dict):
        return {k: _np_tree(v) for k, v in obj.items()}
    return np.asarray(obj)


def _prep(x, params):
    """Host-side input prep: build the 8 per-core constant packs."""
    x = np.asarray(x)
    p = _np_tree(params)

    wavA = x[0, 0, 0]
    wavB = x[0, 0, -1]
    eeg = x[0, 0, 1:-1, TD:WIN - 1]                  # (16, SEQ)
    idx = (np.arange(TD)[:, None] + 1) + np.arange(SEQ)[None, :]
    wA = np.ascontiguousarray(wavA[idx].T)           # (SEQ, 16)
    wB = np.ascontiguousarray(wavB[idx].T)
    eeg_t = np.ascontiguousarray(eeg.T)              # (SEQ, 16)

    def conv_params(i):
        return dict(w1=p["conv1_w"][i], b1=p["conv1_b"][i],
                    w2=p["conv2_w"][i], b2=p["conv2_b"][i],
                    w3=p["conv3_w"][i], b3=p["conv3_b"][i],
                    fc1_w=p["fc1_w"][i], fc1_b=p["fc1_b"][i],
                    fc2_w=p["fc2_w"][i], fc2_b=p["fc2_b"][i])

    # core -> (encoder params, xq, xk, conv branch index)
    plan = [
        (p["a2e"], wA, eeg_t, 0),     # wavA_t  / branch 0
        (p["e2a"], eeg_t, wA, 1),     # eegA    / branch 1
        (p["e2a2"], eeg_t, wB, 2),    # eegB    / branch 2
        (p["a2e2"], wB, eeg_t, 3),    # wavB_t  / branch 3
    ]
    packs = [_pack_core(xq, xk, encp, conv_params(bi))
             for encp, xq, xk, bi in plan]
    in_maps = [{"cpack": packs[c % 4]} for c in range(8)]
    return in_maps, wA


def _host_head(feats, p):
    """Tiny 2-layer sigmoid head (1.7 KFLOP) on the gathered features."""
    z = np.concatenate([f.astype(np.float64) for f in feats], axis=1)
    s1 = 1.0 / (1.0 + np.exp(-(z @ p["out_w1"].astype(np.float64)
                               + p["out_b1"].astype(np.float64))))
    o = 1.0 / (1.0 + np.exp(-(s1 @ p["out_w2"].astype(np.float64)
                              + p["out_b2"].astype(np.float64))))
    return o.astype(np.float32)


def kernel(x, params):
    in_maps, wA = _prep(x, params)
    nc = _get_nc()
    res = run_bass_kernel_spmd(nc, in_maps, list(range(8))).results

    p = _np_tree(params)
    out = _host_head([res[c]["feat"] for c in range(4)], p)
    aeA = res[0]["attn"]
    eaA = res[1]["attn"]
    eaB = res[2]["attn"]
    aeB = res[3]["attn"]
    wavA_before = wA[None].astype(np.float32)
    wavA_after = res[0]["enc"][None]
    ae_fcA = np.ascontiguousarray(res[0]["fch"].T)
    ae_fc_lastA = res[0]["fco"]
    return (out, aeA, (wavA_before, wavA_after), ae_fcA, eaA, aeB, eaB,
            ae_fc_lastA)


# revision 28
# speedup vs baseline: 2.0459x; 1.0346x over previous
"""Trainium2 Bass kernel for the MulT-style cross-modal CNN/transformer.

Strategy (SPMD over 8 NeuronCores):
  - The model contains 4 independent 5-layer cross-modal encoders followed by
    4 independent conv branches and a tiny shared head. Core c (c in 0..3)
    runs encoder c and conv branch c on its own encoder output; cores 4..7
    mirror cores 0..3 (same program, same data -> harmless redundancy).
  - Branch features (1,10) are per-core outputs; the 1.7-KFLOP sigmoid head
    runs on the host (collectives cost ~80us of latency for 160 bytes).
  - Host-side prep folds all LayerNorm gains/biases into the adjacent weight
    matrices, folds biases into ones-row augmented matmuls, pre-applies the
    attention output projection to V (vo = v @ Wo + bo), and packs every
    constant this core needs into ONE [128, NCOL] f32 tensor -> one DMA.
  - PE instruction count dominates (fixed ~400ns/matmul at these sizes), so
    the K-side projections of all 5 layers are batched into 32-partition-
    aligned block matmuls, the 3 convs into one 32-aligned block layout, and
    softmax normalization is folded into the residual add (1/rowsum scaling)
    so it leaves the critical path.
  - On-device transcendentals use only the `natural_log_exp_and_others`
    activation table (copy/relu/ln/exp): rsqrt(v) = exp(-0.5*ln(v+eps)) ->
    exactly one ACT table load per run.
  - All SBUF/PSUM access patterns start at partition offsets that are
    multiples of 32 (HW quadrant constraint).
"""

import numpy as np
from contextlib import ExitStack

import concourse.bass as bass
import concourse.tile as tile
from concourse import bacc, mybir
from concourse.bass_utils import run_bass_kernel_spmd

# The act-table chooser statically maps each activation function to the
# first table set containing it (Exp -> exp_and_others, Ln -> natural_log),
# which thrashes ACT_TABLE_LOAD (~1.3us each) on every Ln/Exp alternation.
# Every function this kernel uses (copy/relu/ln/exp) lives in the
# natural_log set, so mask all other sets (preserving dict order, hence
# act_func_set_id indices) to get exactly one table load per run.
_orig_gat = bacc.get_activation_tables


def _gat_single_set(arch):
    out = {}
    for name, fns in _orig_gat(arch).items():
        out[name] = fns if name.startswith("natural_log") else set()
    return out


bacc.get_activation_tables = _gat_single_set

F32 = mybir.dt.float32
AF = mybir.ActivationFunctionType
ALU = mybir.AluOpType
AX = mybir.AxisListType

SEQ, D, L, FF, OC, NB = 123, 16, 5, 64, 10, 4
WIN, TD = 140, 16
KS = (8, 9, 10)            # conv kernel sizes
NTAP = max(KS)             # 10 accumulation taps
EPS = 1e-5
SCL = float(D) ** 0.5      # 4.0
ISC = float(D) ** -0.5     # 0.25
# layer batches for the K-side block matmuls: (start layer, n layers)
BATCH = ((0, 3), (3, 2))

# ------------------------------------------------------------------ layout --
_cols: dict[str, tuple[int, int]] = {}
_ncol = 0


def _alloc(name: str, cols: int) -> None:
    global _ncol
    _cols[name] = (_ncol, cols)
    _ncol += cols


_alloc("ident", SEQ)           # [123,123] identity
_alloc("xq", D)                # [123,16] raw query-stream input
_alloc("xk", D)                # [123,16] raw key-stream input
_alloc("pos", D)               # [123,16] positional embedding
_alloc("lnfg", D)              # [123,16] final-LN gain, broadcast
_alloc("lnfb", D)              # [123,16] final-LN bias, broadcast
for b, (l0, nl) in enumerate(BATCH):
    _alloc(f"wkbB{b}", 32 * nl)            # [17, 32nl]
    _alloc(f"wvbB{b}", 32 * nl)            # [17, 32nl] (+ones-row selectors)
    _alloc(f"wobB{b}", 16 * nl)            # [32nl, 16nl] block-diag
for l in range(L):
    _alloc(f"wqbT{l}", D + 1)  # [16,17] at ROWS 32j (j = in-batch index)
for l in range(L):
    _alloc(f"fc1b{l}", FF + 1)  # [17,65] (fc1';b1') + unit col -> ones row
    _alloc(f"fc2b{l}", D)      # [65,16]
for t in range(NTAP):
    _alloc(f"cw{t}", 96)       # [17,96] conv tap t, conv j at cols 32j
_alloc("lnfgc", 1)            # [17,1] lnf gain col (+1 at row 16)
_alloc("lnfbc", 1)            # [17,1] lnf bias col (+0 at row 16)
_alloc("fc1w97", 30)           # [97,30]: rows 32j+r = fc1_w[10j+r], row96=b
_alloc("fc2w33", OC)           # [33,10]: rows 0-29 fc2_w, row 32 = b
NCOL = _ncol

E16 = np.zeros((17, 1))
E16[16, 0] = 1.0


def _pos_embed() -> np.ndarray:
    half = D // 2
    freqs = np.exp(np.arange(half) * (-np.log(10000.0) / (half - 1)))
    ang = np.arange(SEQ)[:, None] * freqs[None, :]
    return np.concatenate([np.sin(ang), np.cos(ang)], axis=1).astype(np.float32)


def _pack_core(xq: np.ndarray, xk: np.ndarray, enc: dict, conv: dict) -> np.ndarray:
    """Build the [128, NCOL] constants tensor for one core (float64 folding)."""
    cp = np.zeros((128, NCOL), np.float64)

    def put(name: str, arr: np.ndarray) -> None:
        c0, w = _cols[name]
        assert arr.shape[1] == w, (name, arr.shape, w)
        cp[: arr.shape[0], c0:c0 + w] = arr

    put("ident", np.eye(SEQ))
    put("xq", xq.astype(np.float64))
    put("xk", xk.astype(np.float64))
    put("pos", _pos_embed().astype(np.float64))
    put("lnfg", np.broadcast_to(enc["lnf_g"], (SEQ, D)))
    put("lnfb", np.broadcast_to(enc["lnf_b"], (SEQ, D)))

    def fold(l):
        g0, b0 = enc["ln0_g"][l], enc["ln0_b"][l]
        gk, bk_ = enc["ln0k_g"][l], enc["ln0k_b"][l]
        wq, wk, wv, wo = enc["Wq"][l], enc["Wk"][l], enc["Wv"][l], enc["Wo"][l]
        wqb = np.vstack([g0[:, None] * wq * ISC,
                         (b0 @ wq + enc["bq"][l])[None] * ISC])      # [17,16]
        wkb = np.vstack([gk[:, None] * wk,
                         (bk_ @ wk + enc["bk"][l])[None]])           # [17,16]
        wvb = np.vstack([gk[:, None] * wv,
                         (bk_ @ wv + enc["bv"][l])[None]])           # [17,16]
        wob = np.vstack([wo, enc["bo"][l][None]])                    # [17,16]
        return wqb, wkb, wvb, wob

    for b, (l0, nl) in enumerate(BATCH):
        wkbB = np.zeros((17, 32 * nl))
        wvbB = np.zeros((17, 32 * nl))
        wobB = np.zeros((32 * nl, 16 * nl))
        for j in range(nl):
            wqb, wkb, wvb, wob = fold(l0 + j)
            wkbB[:, 32 * j:32 * j + 16] = wkb
            wvbB[:, 32 * j:32 * j + 16] = wvb
            wvbB[16, 32 * j + 16] = 1.0      # selects knT ones row -> vT ones
            wobB[32 * j:32 * j + 17, 16 * j:16 * j + 16] = wob
            # M_l = wqb @ kT_l: lhsT[k,i] = wqb[i,k], placed at rows 32j to
            # match kTall's base partition for the per-layer M matmul
            c0, w = _cols[f"wqbT{l0 + j}"]
            cp[32 * j:32 * j + 16, c0:c0 + w] = wqb.T
        put(f"wkbB{b}", wkbB)
        put(f"wvbB{b}", wvbB)
        put(f"wobB{b}", wobB)

    for l in range(L):
        g1, b1 = enc["ln1_g"][l], enc["ln1_b"][l]
        f1 = np.vstack([g1[:, None] * enc["fc1_w"][l],
                        (b1 @ enc["fc1_w"][l] + enc["fc1_b"][l])[None]])
        put(f"fc1b{l}", np.hstack([f1, E16]))                        # [17,65]
        put(f"fc2b{l}", np.vstack([enc["fc2_w"][l], enc["fc2_b"][l][None]]))

    # conv taps: cw_t[ic, 32j+oc] = w_j[oc, ic, t] (0 when t >= k_j);
    # bias row 16 only on tap 0 (multiplied by the enc ones-row).
    for t in range(NTAP):
        cw = np.zeros((17, 96))
        for j, k in enumerate(KS):
            if t < k:
                cw[:D, 32 * j:32 * j + OC] = conv[f"w{j+1}"][:, :, t].T
            if t == 0:
                cw[D, 32 * j:32 * j + OC] = conv[f"b{j+1}"]
        put(f"cw{t}", cw)
    f97 = np.zeros((97, 30))
    for j in range(3):
        f97[32 * j:32 * j + OC] = conv["fc1_w"][OC * j:OC * j + OC]
    f97[96] = conv["fc1_b"]
    put("fc1w97", f97)
    gcol = np.zeros((17, 1)); gcol[:16, 0] = enc["lnf_g"]; gcol[16, 0] = 1.0
    bcol = np.zeros((17, 1)); bcol[:16, 0] = enc["lnf_b"]
    put("lnfgc", gcol)
    put("lnfbc", bcol)
    f33 = np.zeros((33, OC))
    f33[0:30] = conv["fc2_w"]
    f33[32] = conv["fc2_b"]
    put("fc2w33", f33)
    return np.ascontiguousarray(cp, np.float32)


# ------------------------------------------------------------- device build --
def _ln_block(nc, sb, st, x_ap, sum_ap, out_ap, tag, eps_ap):
    """LayerNorm standardize: out = (x - mean(x)) * rsqrt(var(x) + eps).
    `sum_ap` is the precomputed row-sum of x (captured for free via
    accum_out on the op that materialized x). Gains/biases are folded into
    downstream weights by the host. rsqrt = exp(-0.5*ln(v+eps))."""
    negmean = st.tile([SEQ, 1], F32, tag=f"nm{tag}", name=f"nm{tag}")
    nc.vector.tensor_scalar_mul(negmean[:], sum_ap, -1.0 / D)
    ssq = st.tile([SEQ, 1], F32, tag=f"sq{tag}", name=f"sq{tag}")
    scr2 = sb.tile([SEQ, D], F32, tag="scr", bufs=2, name=f"scr{tag}")
    # DVE: out = (x - mean) * x, accum = sum((x-m)x) = sum((x-m)^2)
    nc.vector.scalar_tensor_tensor(scr2[:], x_ap, negmean[:], x_ap,
                                   op0=ALU.add, op1=ALU.mult,
                                   accum_out=ssq[:])
    lnv = st.tile([SEQ, 1], F32, tag=f"lv{tag}", name=f"lv{tag}")
    nc.scalar.activation(lnv[:], ssq[:], AF.Ln, scale=1.0 / D, bias=eps_ap)
    rstd = st.tile([SEQ, 1], F32, tag=f"rs{tag}", name=f"rs{tag}")
    nc.scalar.activation(rstd[:], lnv[:], AF.Exp, scale=-0.5)
    # DVE: out = (x + negmean) * rstd
    nc.vector.tensor_scalar(out_ap, x_ap, negmean[:], rstd[:],
                            op0=ALU.add, op1=ALU.mult)


def _build_nc():
    nc = bacc.Bacc("TRN2", target_bir_lowering=False, debug=False,
                   num_devices=8)
    cp_d = nc.dram_tensor("cpack", [128, NCOL], F32, kind="ExternalInput")
    attn_d = nc.dram_tensor("attn", [SEQ, SEQ], F32, kind="ExternalOutput")
    fch_d = nc.dram_tensor("fch", [FF, SEQ], F32, kind="ExternalOutput")
    fco_d = nc.dram_tensor("fco", [SEQ, D], F32, kind="ExternalOutput")
    enc_d = nc.dram_tensor("enc", [D, SEQ], F32, kind="ExternalOutput")
    feat_d = nc.dram_tensor("feat", [1, OC], F32, kind="ExternalOutput")

    with tile.TileContext(nc, trace_sim=False) as tc, ExitStack() as ctx:
        const = ctx.enter_context(tc.tile_pool(name="const", bufs=1))
        pers = ctx.enter_context(tc.tile_pool(name="pers", bufs=1))
        sb = ctx.enter_context(tc.tile_pool(name="sb", bufs=2))
        st = ctx.enter_context(tc.tile_pool(name="st", bufs=2))
        ps = ctx.enter_context(tc.tile_pool(name="ps", bufs=3, space="PSUM"))
        psb = ctx.enter_context(tc.tile_pool(name="psb", bufs=2, space="PSUM"))
        pso = ctx.enter_context(tc.tile_pool(name="pso", bufs=2, space="PSUM"))

        cp = const.tile([128, NCOL], F32, tag="cp", name="cp")
        # split the constants DMA: attention-critical columns first so the
        # first layer unblocks before the FFN/conv constants finish loading
        _split = _cols["fc1b0"][0]
        nc.sync.dma_start(cp[:, 0:_split], cp_d.ap()[:, 0:_split])
        nc.sync.dma_start(cp[:, _split:], cp_d.ap()[:, _split:])

        def C(name, rows):
            c0, w = _cols[name]
            return cp[0:rows, c0:c0 + w]

        ident = C("ident", SEQ)

        def MM(out, lhsT, rhs, **kw):
            nc.tensor.matmul(out, lhsT, rhs, **kw)

        def TR(out, in_):
            nc.tensor.transpose(out, in_, ident)

        # persistent standardized-activation tiles with a ones column 16
        # (transposing them yields the ones-augmented [17,123] layout)
        kstd = pers.tile([SEQ, D + 1], F32, tag="kstd", name="kstd")
        xstd = pers.tile([SEQ, D + 1], F32, tag="xstd", name="xstd")
        l1std = pers.tile([SEQ, D + 1], F32, tag="l1std", name="l1std")
        xfstd = pers.tile([SEQ, D + 1], F32, tag="xfstd", name="xfstd")
        for t_ in (kstd, xstd, l1std, xfstd):
            nc.vector.memset(t_[:, D:D + 1], 1.0)
        epsv = pers.tile([SEQ, 1], F32, tag="epsv", name="epsv")
        nc.vector.memset(epsv[:], EPS)
        tvec = pers.tile([97, 1], F32, tag="tvec", name="tvec")
        nc.vector.memset(tvec[:], 0.0)
        nc.vector.memset(tvec[96:97, :], 1.0)
        hvv = pers.tile([33, 1], F32, tag="hvv", name="hvv")
        nc.vector.memset(hvv[:], 0.0)
        nc.vector.memset(hvv[32:33, :], 1.0)

        # x0 = 4*xq + POS ; xk0 = 4*xk + POS (row-sums captured for the LNs)
        x = pers.tile([SEQ, D], F32, tag="x0", name="x0")
        xsum = st.tile([SEQ, 1], F32, tag="xsum", bufs=3, name="xsum0")
        nc.vector.scalar_tensor_tensor(x[:], C("xq", SEQ), SCL, C("pos", SEQ),
                                       op0=ALU.mult, op1=ALU.add,
                                       accum_out=xsum[:])
        xk0 = pers.tile([SEQ, D], F32, tag="xk0", name="xk0")
        ksum = st.tile([SEQ, 1], F32, tag="ksum", bufs=1, name="ksum")
        nc.vector.scalar_tensor_tensor(xk0[:], C("xk", SEQ), SCL,
                                       C("pos", SEQ), op0=ALU.mult,
                                       op1=ALU.add, accum_out=ksum[:])

        # key-stream standardization (constant across layers: LN g/b folded)
        _ln_block(nc, sb, st, xk0[:], ksum[:], kstd[:, 0:D], "k", epsv[:])
        ksT_ps = ps.tile([D + 1, SEQ], F32, tag="psK", name="ksT_ps")
        TR(ksT_ps[:], kstd[:])
        knT = sb.tile([D + 1, SEQ], F32, tag="knT", bufs=1, name="knT")
        nc.scalar.copy(knT[:], ksT_ps[:])

        # ---- K-side projections for ALL layers, batched (off-path) ----
        # kTall rows 32j..+15 = kT of layer l0+j ; Mall rows 32j..+16 = M_l
        # vTall rows 32j..+16 = vT_l (+ones) ; voall cols 16j..+15 = vo_l
        m_lyr, vo_all = [], []
        for b, (l0, nl) in enumerate(BATCH):
            kT_ps = ps.tile([32 * nl, SEQ], F32, tag="psK", name=f"kT_ps{b}")
            MM(kT_ps[:], C(f"wkbB{b}", D + 1), knT[:],
                             start=True, stop=True)
            kT_s = sb.tile([32 * nl, SEQ], F32, tag="kT", name=f"kT_s{b}")
            nc.vector.tensor_copy(kT_s[:], kT_ps[:])
            for j in range(nl):
                c0, w = _cols[f"wqbT{l0 + j}"]
                m_ps = ps.tile([D + 1, SEQ], F32, tag="psK",
                               name=f"m_ps{l0 + j}")
                MM(m_ps[:],
                                 cp[32 * j:32 * j + 16, c0:c0 + w],
                                 kT_s[32 * j:32 * j + 16, :], start=True,
                                 stop=True)
                m_s = sb.tile([D + 1, SEQ], F32, tag="m", bufs=5,
                              name=f"m_s{l0 + j}")
                nc.vector.tensor_copy(m_s[:], m_ps[:])
                m_lyr.append(m_s)

            vT_ps = ps.tile([32 * nl, SEQ], F32, tag="psK", name=f"vT_ps{b}")
            MM(vT_ps[:], C(f"wvbB{b}", D + 1), knT[:],
                             start=True, stop=True)
            vT_s = sb.tile([32 * nl, SEQ], F32, tag="vT", name=f"vT_s{b}")
            nc.scalar.copy(vT_s[:], vT_ps[:])
            vo_ps = pso.tile([SEQ, 16 * nl], F32, tag="pso", name=f"vo_ps{b}")
            MM(vo_ps[:], vT_s[:], C(f"wobB{b}", 32 * nl),
                             start=True, stop=True)
            vo_s = sb.tile([SEQ, 16 * nl], F32, tag="vo", name=f"vo_s{b}")
            nc.scalar.copy(vo_s[:], vo_ps[:])
            vo_all.append(vo_s)

        attn_keep = None
        fco_s = None
        fch_keep = None
        for l in range(L):
            b, j = (0, l) if l < 3 else (1, l - 3)
            # ---- critical path: Q-side LN -> scores -> softmax -> out ----
            _ln_block(nc, sb, st, x[:], xsum[:], xstd[:, 0:D], f"q{l}",
                      epsv[:])
            xsT_ps = ps.tile([D + 1, SEQ], F32, tag="psK", name=f"xsT_ps{l}")
            TR(xsT_ps[:], xstd[:])
            xsT_s = sb.tile([D + 1, SEQ], F32, tag="xsT", name=f"xsT_s{l}")
            nc.vector.tensor_copy(xsT_s[:], xsT_ps[:])
            sc_ps = psb.tile([SEQ, SEQ], F32, tag="big", name=f"sc_ps{l}")
            MM(sc_ps[:], xsT_s[:], m_lyr[l][:], start=True,
                             stop=True)
            # softmax: no max-subtraction (scores are O(1); shift-invariant),
            # normalization folded into the residual add via 1/rowsum
            rowsum = st.tile([SEQ, 1], F32, tag="rsum", name=f"rowsum{l}")
            e_s = sb.tile([SEQ, SEQ], F32, tag="e", name=f"e_s{l}")
            nc.scalar.activation(e_s[:], sc_ps[:], AF.Exp)
            nc.vector.reduce_sum(rowsum[:], e_s[:], axis=AX.X)
            rinv = st.tile([SEQ, 1], F32, tag="rinv", name=f"rinv{l}")
            nc.vector.reciprocal(rinv[:], rowsum[:])
            eT_ps = psb.tile([SEQ, SEQ], F32, tag="big", name=f"eT_ps{l}")
            TR(eT_ps[:], e_s[:])
            eT_s = sb.tile([SEQ, SEQ], F32, tag="eT", name=f"eT_s{l}")
            nc.vector.tensor_copy(eT_s[:], eT_ps[:])
            o_ps = pso.tile([SEQ, D], F32, tag="pso", name=f"o_ps{l}")
            MM(o_ps[:], eT_s[:],
                             vo_all[b][:, 16 * j:16 * j + 16], start=True,
                             stop=True)
            x2 = sb.tile([SEQ, D], F32, tag="x", bufs=3, name=f"x2_{l}")
            x2sum = st.tile([SEQ, 1], F32, tag="xsum", bufs=3,
                            name=f"x2sum{l}")
            # x2 = (E @ vo) * (1/rowsum) + x  == attn @ vo + x
            nc.vector.scalar_tensor_tensor(x2[:], o_ps[:], rinv[:], x[:],
                                           op0=ALU.mult, op1=ALU.add,
                                           accum_out=x2sum[:])
            if l == L - 1:
                attn_keep = sb.tile([SEQ, SEQ], F32, tag="attn", bufs=1,
                                    name="attn_keep")
                nc.vector.tensor_scalar_mul(attn_keep[:], e_s[:], rinv[:])

            # ---- FFN ----
            _ln_block(nc, sb, st, x2[:], x2sum[:], l1std[:, 0:D], f"f{l}",
                      epsv[:])
            l1T_ps = ps.tile([D + 1, SEQ], F32, tag="psK", name=f"l1T_ps{l}")
            TR(l1T_ps[:], l1std[:])
            l1T_s = sb.tile([D + 1, SEQ], F32, tag="l1T", name=f"l1T_s{l}")
            nc.vector.tensor_copy(l1T_s[:], l1T_ps[:])
            hT_ps = pso.tile([FF + 1, SEQ], F32, tag="pso", name=f"hT_ps{l}")
            MM(hT_ps[:], C(f"fc1b{l}", D + 1), l1T_s[:],
                             start=True, stop=True)
            hT_s = sb.tile([FF + 1, SEQ], F32, tag="hT", name=f"hT_s{l}")
            nc.scalar.activation(hT_s[:], hT_ps[:], AF.Relu)
            fco_ps = pso.tile([SEQ, D], F32, tag="pso", name=f"fco_ps{l}")
            MM(fco_ps[:], hT_s[:], C(f"fc2b{l}", FF + 1),
                             start=True, stop=True)
            if l == L - 1:
                fco_s = sb.tile([SEQ, D], F32, tag="fcos", bufs=1,
                                name="fco_s")
                nc.vector.tensor_copy(fco_s[:], fco_ps[:])
                fch_keep = hT_s
            x3 = sb.tile([SEQ, D], F32, tag="x", bufs=3, name=f"x3_{l}")
            xsum = st.tile([SEQ, 1], F32, tag="xsum", bufs=3,
                           name=f"x3sum{l}")
            nc.vector.scalar_tensor_tensor(x3[:], fco_ps[:], 1.0, x2[:],
                                           op0=ALU.mult, op1=ALU.add,
                                           accum_out=xsum[:])
            x = x3

        # ---- final LN, then transpose and apply gain/bias in one fused
        # tensor_scalar during the PSUM->SBUF copy (encT orientation puts
        # the gain/bias along partitions); enc is output transposed and
        # un-transposed on the host
        _ln_block(nc, sb, st, x[:], xsum[:], xfstd[:, 0:D], "fin", epsv[:])
        encT_ps = ps.tile([D + 1, SEQ], F32, tag="psK", name="encT_ps")
        TR(encT_ps[:], xfstd[:])
        encT_s = sb.tile([D + 1, SEQ + NTAP - 1], F32, tag="encT", bufs=1,
                         name="encT_s")
        nc.vector.memset(encT_s[:, SEQ:], 0.0)
        nc.vector.tensor_scalar(encT_s[:, 0:SEQ], encT_ps[:],
                                C("lnfgc", D + 1), C("lnfbc", D + 1),
                                op0=ALU.mult, op1=ALU.add)

        # encoder outputs -> DRAM
        nc.sync.dma_start(attn_d.ap(), attn_keep[:])
        nc.sync.dma_start(fch_d.ap(), fch_keep[0:FF, :])
        nc.sync.dma_start(fco_d.ap(), fco_s[:])
        nc.sync.dma_start(enc_d.ap(), encT_s[0:D, 0:SEQ])
        # conv j output rows at 32j..32j+9 (quadrant-aligned)
        cv_ps = psb.tile([96, SEQ], F32, tag="big", name="cv_ps")
        for t in range(NTAP):
            MM(cv_ps[:], C(f"cw{t}", D + 1),
                             encT_s[:, t:t + SEQ], start=(t == 0),
                             stop=(t == NTAP - 1))
        # max over valid windows straight from PSUM; relu after max
        # (relu(max(x)) == max(relu(x)) since relu is monotone)
        for j, k in enumerate(KS):
            nc.vector.reduce_max(tvec[32 * j:32 * j + OC, :],
                                 cv_ps[32 * j:32 * j + OC, 0:SEQ - k + 1],
                                 axis=AX.X)
        tvr = sb.tile([97, 1], F32, tag="tvr", bufs=1, name="tvr")
        nc.scalar.activation(tvr[:], tvec[:], AF.Relu)
        # branch fc1 (bias via ones row 96): ftT = fc1w97.T @ tvr [30,1]
        ftT_ps = pso.tile([30, 1], F32, tag="pso", name="ftT_ps")
        MM(ftT_ps[:], C("fc1w97", 97), tvr[:], start=True,
                         stop=True)
        nc.scalar.activation(hvv[0:30, :], ftT_ps[:], AF.Relu)
        feat_ps = pso.tile([1, OC], F32, tag="pso", name="feat_ps")
        MM(feat_ps[:], hvv[:], C("fc2w33", 33), start=True,
                         stop=True)
        feat_s = sb.tile([1, OC], F32, tag="feat", bufs=1, name="feat_s")
        nc.vector.tensor_copy(feat_s[:], feat_ps[:])
        nc.sync.dma_start(feat_d.ap(), feat_s[:])

    nc.compile()
    return nc


_NC_CACHE: list = []


def _get_nc():
    if not _NC_CACHE:
        _NC_CACHE.append(_build_nc())
    return _NC_CACHE[0]


# ------------------------------------------------------------------- driver --
def _np_tree(obj):
    if isinstance(obj, dict):
        return {k: _np_tree(v) for k, v in obj.items()}
    return np.asarray(obj)


def _prep(x, params):
    """Host-side input prep: build the 8 per-core constant packs."""
    x = np.asarray(x)
    p = _np_tree(params)

    wavA = x[0, 0, 0]
    wavB = x[0, 0, -1]
    eeg = x[0, 0, 1:-1, TD:WIN - 1]                  # (16, SEQ)
    idx = (np.arange(TD)[:, None] + 1) + np.arange(SEQ)[None, :]
    wA = np.ascontiguousarray(wavA[idx].T)           # (SEQ, 16)
    wB = np.ascontiguousarray(wavB[idx].T)
    eeg_t = np.ascontiguousarray(eeg.T)              # (SEQ, 16)

    def conv_params(i):
        return dict(w1=p["conv1_w"][i], b1=p["conv1_b"][i],
                    w2=p["conv2_w"][i], b2=p["conv2_b"][i],
                    w3=p["conv3_w"][i], b3=p["conv3_b"][i],
                    fc1_w=p["fc1_w"][i], fc1_b=p["fc1_b"][i],
                    fc2_w=p["fc2_w"][i], fc2_b=p["fc2_b"][i])

    # core -> (encoder params, xq, xk, conv branch index)
    plan = [
        (p["a2e"], wA, eeg_t, 0),     # wavA_t  / branch 0
        (p["e2a"], eeg_t, wA, 1),     # eegA    / branch 1
        (p["e2a2"], eeg_t, wB, 2),    # eegB    / branch 2
        (p["a2e2"], wB, eeg_t, 3),    # wavB_t  / branch 3
    ]
    packs = [_pack_core(xq, xk, encp, conv_params(bi))
             for encp, xq, xk, bi in plan]
    in_maps = [{"cpack": packs[c % 4]} for c in range(8)]
    return in_maps, wA


def _host_head(feats, p):
    """Tiny 2-layer sigmoid head (1.7 KFLOP) on the gathered features."""
    z = np.concatenate([f.astype(np.float64) for f in feats], axis=1)
    s1 = 1.0 / (1.0 + np.exp(-(z @ p["out_w1"].astype(np.float64)
                               + p["out_b1"].astype(np.float64))))
    o = 1.0 / (1.0 + np.exp(-(s1 @ p["out_w2"].astype(np.float64)
                              + p["out_b2"].astype(np.float64))))
    return o.astype(np.float32)


def kernel(x, params):
    in_maps, wA = _prep(x, params)
    nc = _get_nc()
    res = run_bass_kernel_spmd(nc, in_maps, list(range(8))).results

    p = _np_tree(params)
    out = _host_head([res[c]["feat"] for c in range(4)], p)
    aeA = res[0]["attn"]
    eaA = res[1]["attn"]
    eaB = res[2]["attn"]
    aeB = res[3]["attn"]
    wavA_before = wA[None].astype(np.float32)
    wavA_after = np.ascontiguousarray(res[0]["enc"].T)[None]
    ae_fcA = np.ascontiguousarray(res[0]["fch"].T)
    ae_fc_lastA = res[0]["fco"]
    return (out, aeA, (wavA_before, wavA_after), ae_fcA, eaA, aeB, eaB,
            ae_fc_lastA)
